# revision 1
# baseline (speedup 1.0000x reference)
"""Trainium2 Bass kernel for nn_DiffAttention (GNN message passing).

Math (per edge i: src s_i -> dst n, dst sorted):
  d_i = (h_dst[n] - h_src[s_i]) @ W_fc.T ;  e_i = tanh(d_i @ w_attn)
  alpha = segment_softmax(e, dst);  out[n] = elu(sum_i alpha_i d_i)
Since e in [-1,1], softmax needs no max-subtraction:
  out[n] = elu(p_dst[n] - (sum_i w_i p_src[s_i]) / (sum_i w_i)),
  w_i = exp(tanh(q_dst[n] - q_src[s_i])), p = h @ W_fc.T, q = p @ w_attn.

Device strategy (8 cores, SPMD, edge-parallel by dst range):
  - node table [NPAD, 132] f32 rows [p_src|1|q_src|p_dst|0|q_dst], built
    sharded (1/8 per core) on PE, then AllGather.
  - per window (<=128 consecutive dst nodes, 16x128 edge slots):
    1 indirect row-gather of the window's node rows (p_dst + q_dst),
    16 indirect row-gathers of per-edge src rows,
    per tile: one-hot S01[edge,slot] = (iota == dst_local) on DVE,
    qd per edge = rowsum(S01 * qb) (qb = q_win broadcast via K=1 matmul),
    w = exp(tanh(qd - qs)) on ACT, rhs = w*[p_src|1] (ACT scale),
    PSUM[slot, 0:65] += S01.T @ rhs  accumulates [sum w*p | sum w].
  - epilogue per window: elu(p_dst - swp/sw) with zero-edge masking.
Host does only index prep (windowing, padding, int16/f32 casts) and
reassembles per-window 128-slot outputs into node order.
"""
import sys
sys.path.insert(0, "/opt/trn_rl_repo")
import numpy as np

N = 100000
D = 64
NC = 8
K = 16            # 128-edge tiles per window
WE = K * 128      # edge slots per window
WIN_NODES = 128
SHARD = 12544     # 98*128 rows built per core
NPAD = NC * SHARD # 100352
DUMMY = N         # zero row (h padded with zeros)
ROW = 132         # [p_src(0:64) | 1(64) | q_src(65) | p_dst(66:130) | 0(130) | q_dst(131)]
MAIN_REPEAT = 1   # test.py overrides for timing
PAY_BUFS = 16


# ---------------------------------------------------------------- host prep
def _partition_edges(dst):
    E = dst.shape[0]
    bounds, e_prev, n_prev = [], 0, 0
    for c in range(1, NC):
        s = (E * c) // NC
        while 0 < s < E and dst[s] == dst[s - 1]:
            s += 1
        node_split = int(dst[s]) if s < E else N
        bounds.append((e_prev, s, n_prev, node_split))
        e_prev, n_prev = s, node_split
    bounds.append((e_prev, E, n_prev, N))
    return bounds


def _build_windows(src, dst, e_lo, e_hi, n_lo, n_hi):
    counts = np.bincount(dst[e_lo:e_hi] - n_lo, minlength=n_hi - n_lo)
    assert counts.max() <= WE, f"node degree {counts.max()} > window capacity"
    starts = np.concatenate([[0], np.cumsum(counts)])
    srcs, dls, bases, nns = [], [], [], []
    n, n_total = 0, n_hi - n_lo
    while n < n_total:
        n_end = min(n + WIN_NODES, n_total)
        while starts[n_end] - starts[n] > WE:
            n_end -= 1
        ecnt = int(starts[n_end] - starts[n])
        elo = e_lo + int(starts[n])
        s = np.full(WE, DUMMY, np.int32)
        dl = np.full(WE, -1.0, np.float32)
        s[:ecnt] = src[elo:elo + ecnt]
        dl[:ecnt] = (dst[elo:elo + ecnt] - (n_lo + n)).astype(np.float32)
        srcs.append(s); dls.append(dl)
        bases.append(n_lo + n); nns.append(n_end - n)
        n = n_end
    return np.stack(srcs), np.stack(dls), np.array(bases), np.array(nns)


def _prep(src, dst):
    src = np.asarray(src, np.int64)
    dst = np.asarray(dst, np.int64)
    if np.any(np.diff(dst) < 0):  # tolerate unsorted edges
        order = np.argsort(dst, kind="stable")
        src, dst = src[order], dst[order]
    bounds = _partition_edges(dst)
    per_core = [_build_windows(src, dst, *b) for b in bounds]
    nW = max(p[0].shape[0] for p in per_core)
    cores = []
    for (s, dl, base, nn) in per_core:
        pad = nW - s.shape[0]
        if pad:
            s = np.concatenate([s, np.full((pad, WE), DUMMY, np.int32)])
            dl = np.concatenate([dl, np.full((pad, WE), -1.0, np.float32)])
            base = np.concatenate([base, np.full(pad, N, np.int64)])
            nn = np.concatenate([nn, np.zeros(pad, np.int64)])
        slot = base[:, None] + np.arange(WIN_NODES)[None, :]
        slot = np.where(np.arange(WIN_NODES)[None, :] < nn[:, None], slot, DUMMY)
        cores.append(dict(src=s, dst_local=dl, base=base, nn=nn,
                          slot_ids=slot.astype(np.int32)))
    return cores, nW


def _to_tiles(a):  # [nW, WE] -> [nW, 128, K]; [w,p,k] = edge k*128+p
    nW = a.shape[0]
    return np.ascontiguousarray(a.reshape(nW, K, 128).transpose(0, 2, 1))


# ---------------------------------------------------------------- device
def _build_program(nW, main_repeat, ablate=""):
    from concourse import bass, bacc, mybir, tile
    f32, i32, i16 = mybir.dt.float32, mybir.dt.int32, mybir.dt.int16

    nc = bacc.Bacc("TRN2", target_bir_lowering=False, debug=False,
                   num_devices=NC)
    hs_e = nc.dram_tensor("hs", [SHARD, D], f32, kind="ExternalInput")
    hd_e = nc.dram_tensor("hd", [SHARD, D], f32, kind="ExternalInput")
    wfc_e = nc.dram_tensor("wfc", [D, D], f32, kind="ExternalInput")
    wat_e = nc.dram_tensor("wat", [D, 1], f32, kind="ExternalInput")
    sidx_e = nc.dram_tensor("sidx", [nW, 128, K], i32, kind="ExternalInput")
    dloc_e = nc.dram_tensor("dloc", [nW, 128, K], f32, kind="ExternalInput")
    nid_e = nc.dram_tensor("nid", [nW, 128, 1], i32, kind="ExternalInput")
    res_e = nc.dram_tensor("res", [nW * 128, D], f32, kind="ExternalOutput")

    with tile.TileContext(nc) as tc:
        with tc.tile_pool(name="c", bufs=1) as cp, \
             tc.tile_pool(name="sb", bufs=3) as sp, \
             tc.tile_pool(name="dr", bufs=1, space="DRAM") as dp:
            pp = tc.alloc_tile_pool(name="psb", bufs=1, space="PSUM")
            # ---- constants (shipped in the NEFF, no gpsimd custom ops)
            ident_d = nc.inline_tensor(np.eye(128, dtype=np.float32),
                                       name="ident_c")
            iota_d = nc.inline_tensor(
                np.tile(np.arange(128, dtype=np.float32), (128, 1)),
                name="iota_c")
            ident = cp.tile([128, 128], f32)
            nc.sync.dma_start(out=ident[:], in_=ident_d[:])
            iotaf = cp.tile([128, 128], f32)
            nc.sync.dma_start(out=iotaf[:], in_=iota_d[:])
            ones_row = cp.tile([1, 128], f32)
            nc.vector.memset(ones_row[:], 1.0)
            ones_col = cp.tile([128, 1], f32)
            nc.vector.memset(ones_col[:], 1.0)

            # ---- weight prep: rhs_build [64, 66] = [W.T | 0 | W.T @ w_attn]
            wfc = cp.tile([D, D], f32)
            nc.sync.dma_start(out=wfc[:], in_=wfc_e[:])
            wat = cp.tile([D, 1], f32)
            nc.sync.dma_start(out=wat[:], in_=wat_e[:])
            wt_ps = pp.tile([D, D], f32, space="PSUM")
            nc.tensor.transpose(out=wt_ps[:], in_=wfc[:], identity=ident[:D, :D])
            v_ps = pp.tile([D, 1], f32, space="PSUM")
            nc.tensor.matmul(out=v_ps[:], lhsT=wfc[:], rhs=wat[:],
                             start=True, stop=True)
            rhsb = cp.tile([D, 66], f32)
            nc.vector.memset(rhsb[:], 0.0)
            nc.vector.tensor_copy(rhsb[:, 0:64], wt_ps[:])
            nc.vector.tensor_copy(rhsb[:, 65:66], v_ps[:])

            # ---- table build (this core's shard)
            tbl_sh = dp.tile([SHARD, ROW], f32)
            for j in range(SHARD // 128):
                r0 = j * 128
                hs = sp.tile([128, D], f32, tag="bh")
                nc.sync.dma_start(out=hs[:], in_=hs_e[r0:r0 + 128, :])
                hd = sp.tile([128, D], f32, tag="bh2")
                nc.sync.dma_start(out=hd[:], in_=hd_e[r0:r0 + 128, :])
                hsT_ps = pp.tile([D, 128], f32, space="PSUM", tag="bt")
                nc.tensor.transpose(out=hsT_ps[:], in_=hs[:], identity=ident[:])
                hsT = sp.tile([D, 128], f32, tag="bs")
                nc.vector.tensor_copy(hsT[:], hsT_ps[:])
                hdT_ps = pp.tile([D, 128], f32, space="PSUM", tag="bt2")
                nc.tensor.transpose(out=hdT_ps[:], in_=hd[:], identity=ident[:])
                hdT = sp.tile([D, 128], f32, tag="bs2")
                nc.vector.tensor_copy(hdT[:], hdT_ps[:])
                pb = pp.tile([128, ROW], f32, space="PSUM", tag="bp")
                nc.tensor.matmul(out=pb[:, 0:66], lhsT=hsT[:], rhs=rhsb[:],
                                 start=True, stop=True)
                nc.tensor.matmul(out=pb[:, 66:132], lhsT=hdT[:], rhs=rhsb[:],
                                 start=True, stop=True)
                tb = sp.tile([128, ROW], f32, tag="bo")
                nc.vector.tensor_copy(tb[:], pb[:])
                nc.vector.memset(tb[:, 64:65], 1.0)
                nc.sync.dma_start(out=tbl_sh[r0:r0 + 128, :], in_=tb[:])

            pp.release()
            pp2 = tc.alloc_tile_pool(name="psm", bufs=2, space="PSUM")

            # ---- all-gather the table
            table = dp.tile([NPAD, ROW], f32)
            nc.gpsimd.collective_compute(
                "AllGather", mybir.AluOpType.bypass,
                replica_groups=[list(range(NC))],
                ins=[tbl_sh.opt()], outs=[table.opt()])

            # ---- main loop
            rep_ctx = tc.For_i(0, main_repeat, 1) if main_repeat > 1 else None
            if rep_ctx is not None:
                rep_ctx.__enter__()
            for w in range(nW):
                sidx = sp.tile([128, K], i32, tag="si")
                nc.sync.dma_start(out=sidx[:], in_=sidx_e[w])
                dloc = sp.tile([128, K], f32, tag="dl")
                nc.sync.dma_start(out=dloc[:], in_=dloc_e[w])
                nid = sp.tile([128, 1], i32, tag="ni")
                nc.sync.dma_start(out=nid[:], in_=nid_e[w])
                nrows = sp.tile([128, ROW], f32, tag="nr")
                nc.gpsimd.indirect_dma_start(
                    out=nrows[:], out_offset=None, in_=table[:],
                    in_offset=bass.IndirectOffsetOnAxis(ap=nid[:], axis=0))
                # qb[p, n] = q_dst of window node n (broadcast to all p)
                qT_ps = pp2.tile([1, 128], f32, space="PSUM", tag="qt")
                nc.tensor.transpose(out=qT_ps[:], in_=nrows[:, 131:132],
                                    identity=ident[:])
                qrow = sp.tile([1, 128], f32, tag="qr")
                nc.vector.tensor_copy(qrow[:], qT_ps[:])
                qb_ps = pp2.tile([128, 128], f32, space="PSUM", tag="qb")
                nc.tensor.matmul(out=qb_ps[:], lhsT=ones_row[:], rhs=qrow[:],
                                 start=True, stop=True)
                qb = sp.tile([128, 128], f32, tag="qbs")
                nc.vector.tensor_copy(qb[:], qb_ps[:])

                acc = pp2.tile([128, 65], f32, space="PSUM", tag="acc")
                for k in range(K):
                    if ablate != "compute_only":
                        pay = sp.tile([128, ROW], f32, tag="pay", bufs=PAY_BUFS)
                        nc.gpsimd.indirect_dma_start(
                            out=pay[:], out_offset=None, in_=table[:],
                            in_offset=bass.IndirectOffsetOnAxis(
                                ap=sidx[:, k:k + 1], axis=0))
                    else:
                        pay = sp.tile([128, ROW], f32, tag="pay", bufs=6)
                        nc.vector.tensor_copy(pay[:], nrows[:])
                    if ablate == "gather_only":
                        continue
                    S01 = sp.tile([128, 128], f32, tag="s01", bufs=8)
                    nc.vector.tensor_scalar(
                        out=S01[:], in0=iotaf[:], scalar1=dloc[:, k:k + 1],
                        scalar2=None, op0=mybir.AluOpType.is_equal)
                    scr = sp.tile([128, 128], f32, tag="scr", bufs=4)
                    nc.vector.tensor_tensor(scr[:], S01[:], qb[:],
                                            op=mybir.AluOpType.mult)
                    qd = sp.tile([128, 1], f32, tag="qd", bufs=8)
                    nc.vector.tensor_reduce(
                        out=qd[:], in_=scr[:], axis=mybir.AxisListType.X,
                        op=mybir.AluOpType.add)
                    th = sp.tile([128, 1], f32, tag="th", bufs=8)
                    nc.scalar.activation(
                        out=th[:], in_=pay[:, 65:66],
                        func=mybir.ActivationFunctionType.Tanh,
                        bias=qd[:], scale=-1.0)
                    wc = sp.tile([128, 1], f32, tag="wc", bufs=8)
                    nc.scalar.activation(
                        out=wc[:], in_=th[:],
                        func=mybir.ActivationFunctionType.Exp)
                    sc = sp.tile([128, 65], f32, tag="sc", bufs=8)
                    nc.scalar.activation(
                        out=sc[:], in_=pay[:, 0:65],
                        func=mybir.ActivationFunctionType.Copy,
                        scale=wc[:])
                    nc.tensor.matmul(out=acc[:], lhsT=S01[:], rhs=sc[:],
                                     start=(k == 0), stop=(k == K - 1))

                # epilogue: out = elu(p_dst - swp/sw) * (sw != 0)
                if ablate == "gather_only":
                    nc.tensor.matmul(out=acc[:], lhsT=ident[:],
                                     rhs=pay[:, 0:65], start=True, stop=True)
                z = sp.tile([128, 1], f32, tag="z")
                nc.vector.tensor_scalar(
                    out=z[:], in0=acc[:, 64:65], scalar1=0.0, scalar2=None,
                    op0=mybir.AluOpType.is_equal)
                den = sp.tile([128, 1], f32, tag="den")
                nc.vector.tensor_tensor(den[:], acc[:, 64:65], z[:],
                                        op=mybir.AluOpType.add)
                rec = sp.tile([128, 1], f32, tag="rec")
                nc.vector.reciprocal(rec[:], den[:])
                nzm = sp.tile([128, 1], f32, tag="nzm")
                nc.vector.scalar_tensor_tensor(
                    out=nzm[:], in0=z[:], scalar=-1.0, in1=ones_col[:],
                    op0=mybir.AluOpType.mult, op1=mybir.AluOpType.add)
                mean = sp.tile([128, D], f32, tag="mean")
                nc.vector.tensor_scalar(
                    out=mean[:], in0=acc[:, 0:64], scalar1=rec[:],
                    scalar2=None, op0=mybir.AluOpType.mult)
                diff = sp.tile([128, D], f32, tag="diff")
                nc.vector.tensor_tensor(diff[:], nrows[:, 66:130], mean[:],
                                        op=mybir.AluOpType.subtract)
                dm = sp.tile([128, D], f32, tag="dm")
                nc.vector.tensor_scalar(
                    out=dm[:], in0=diff[:], scalar1=nzm[:], scalar2=None,
                    op0=mybir.AluOpType.mult)
                neg = sp.tile([128, D], f32, tag="neg")
                nc.vector.tensor_scalar(
                    out=neg[:], in0=dm[:], scalar1=0.0, scalar2=None,
                    op0=mybir.AluOpType.min)
                ex = sp.tile([128, D], f32, tag="ex")
                nc.scalar.activation(out=ex[:], in_=neg[:],
                                     func=mybir.ActivationFunctionType.Exp)
                pos = sp.tile([128, D], f32, tag="pos")
                nc.vector.tensor_scalar(
                    out=pos[:], in0=dm[:], scalar1=0.0, scalar2=None,
                    op0=mybir.AluOpType.max)
                res = sp.tile([128, D], f32, tag="res")
                nc.vector.scalar_tensor_tensor(
                    out=res[:], in0=ex[:], scalar=-1.0, in1=pos[:],
                    op0=mybir.AluOpType.add, op1=mybir.AluOpType.add)
                nc.sync.dma_start(out=res_e[w * 128:(w + 1) * 128, :],
                                  in_=res[:])
            if rep_ctx is not None:
                rep_ctx.__exit__(None, None, None)
            pp2.release()
    nc.compile()
    return nc


_CACHE = {}


def _get_program(nW, main_repeat, ablate=""):
    key = (nW, main_repeat, ablate)
    if key not in _CACHE:
        _CACHE[key] = _build_program(nW, main_repeat, ablate)
    return _CACHE[key]


def kernel(h_src, h_dst, W_fc, w_attn, src, dst, _main_repeat=MAIN_REPEAT,
           _return_walls=False, _ablate=""):
    from concourse.bass_utils import run_bass_kernel_spmd

    h_src = np.ascontiguousarray(np.asarray(h_src, np.float32))
    h_dst = np.ascontiguousarray(np.asarray(h_dst, np.float32))
    W_fc = np.ascontiguousarray(np.asarray(W_fc, np.float32))
    w_attn = np.ascontiguousarray(np.asarray(w_attn, np.float32)).reshape(D, 1)
    cores, nW = _prep(src, dst)

    hp = np.zeros((NPAD, D), np.float32); hp[:N] = h_src
    hq = np.zeros((NPAD, D), np.float32); hq[:N] = h_dst

    in_maps = []
    for c, core in enumerate(cores):
        in_maps.append({
            "hs": hp[c * SHARD:(c + 1) * SHARD],
            "hd": hq[c * SHARD:(c + 1) * SHARD],
            "wfc": W_fc,
            "wat": w_attn,
            "sidx": _to_tiles(core["src"]),
            "dloc": _to_tiles(core["dst_local"]),
            "nid": core["slot_ids"][:, :, None],
            })
    nc = _get_program(nW, _main_repeat, _ablate)
    import time
    walls = []
    t0 = time.time()
    res = run_bass_kernel_spmd(nc, in_maps, list(range(NC)))
    walls.append(time.time() - t0)

    out = np.zeros((N, D), np.float32)
    for c, core in enumerate(cores):
        r = res.results[c]["res"].reshape(nW, 128, D)
        base, nn = core["base"], core["nn"]
        for w in range(nW):
            if nn[w] > 0:
                out[base[w]:base[w] + nn[w]] = r[w, :nn[w]]
    if _return_walls:
        return out, walls
    return out


if __name__ == "__main__":
    d = np.load("/root/problem/refdata.npz")
    out = kernel(d["h_src"], d["h_dst"], d["W_fc"], d["w_attn"],
                 d["src"], d["dst"])
    exp = d["expected"]
    rel = np.linalg.norm(out - exp) / np.linalg.norm(exp)
    print(f"rel_l2 = {rel:.3e}  maxabs = {np.abs(out - exp).max():.3e}")



# revision 4
# speedup vs baseline: 1.3889x; 1.3889x over previous
"""Trainium2 Bass kernel for nn_DiffAttention — node-major 4-pass dma_gather.

Math (edge i: src s -> dst n, per-dst softmax over incoming edges):
  p = h @ W_fc.T ; q = p @ w_attn ; w_i = exp(tanh(q_dst[n] - q_src[s]))
  out[n] = elu(p_dst[n] - (sum_i w_i p_src[s_i]) / (sum_i w_i))
(e = tanh(..) in [-1,1] so softmax max-subtraction is unnecessary.)

Device strategy (8 cores, SPMD, dst-sharded 12544 nodes/core):
  - fp16 src table [100352, 128]: rows [p(64)|one|q|junk], node id permuted
    into 4 chunks of 25088 rows (25087 real + 1 zero dummy) so every
    dma_gather idx fits in int16. Built sharded on PE, AllGather'd.
  - 4 passes per core: pass k covers edges with src in chunk k. Nodes are
    re-sorted by pass-degree; groups of <=8 subwindows x 128 nodes share a
    uniform per-node slot count F. One dma_gather per group fetches all
    edge rows node-major: slot j -> partition j%128, block j//128 = (sub,f).
    Per-edge w on ACT (qd is per-partition!), weighted rows by in-place DVE
    mult, per-node sums by DVE reduce along f. Partials [swp|sw] -> fp16
    tables in pass order.
  - Combine: per pass one dma_gather re-orders partials to canonical node
    layout (p=n//98, col=n%98); sum, then batched epilogue
    elu(p_dst - swp/sw) with zero-degree masking; one plain DMA out.
Host does only index prep (degree sorts, idx arrays, permuted h copies).
"""
import sys
sys.path.insert(0, "/opt/trn_rl_repo")
import numpy as np

N = 100000
D = 64
NC = 8
SHARD = 12544            # nodes per core = 128 * 98
COLS = 98
RCH = 25087              # real nodes per chunk
CROWS = 25088            # table rows per chunk (last row zero dummy)
NPAD = CROWS * 4         # 100352
NCHUNK = 4
ELEM = 128               # fp16 elems per src-table row (256B)
MAXSLOT = 32             # max nsub*F per gather group (pay tile 8KB/part)
MAXSUB = 8
MAIN_REPEAT = 1


# ---------------------------------------------------------------- host prep
def _wrap_idx(flat):
    """[n] int -> [128, n//16] int16, idx j at [16s + j%16, j//16] stripes
    replicated (HW SWDGE reads stripe 16:32; interp reads 0:16)."""
    w = flat.reshape(-1, 16).T
    return np.ascontiguousarray(np.tile(w, (8, 1)).astype(np.int16))


def _prep(src, dst):
    src = np.asarray(src, np.int64)
    dst = np.asarray(dst, np.int64)
    if np.any(np.diff(dst) < 0):
        o = np.argsort(dst, kind="stable")
        src, dst = src[o], dst[o]
    per_core = []
    for c in range(NC):
        n_lo = c * SHARD
        e_lo = np.searchsorted(dst, n_lo)
        e_hi = np.searchsorted(dst, min(n_lo + SHARD, N))
        s = src[e_lo:e_hi]
        d = dst[e_lo:e_hi] - n_lo
        passes = []
        for k in range(NCHUNK):
            m = (s // RCH) == k
            sk = (s[m] % RCH).astype(np.int64)
            dk = d[m]
            deg = np.bincount(dk, minlength=SHARD)
            order = np.argsort(-deg, kind="stable")
            rank = np.empty(SHARD, np.int64)
            rank[order] = np.arange(SHARD)
            eo = np.argsort(rank[dk], kind="stable")
            sk = sk[eo]
            cnt = deg[order]
            starts = np.concatenate([[0], np.cumsum(cnt)])
            passes.append(dict(cnt=cnt, starts=starts, sk=sk,
                               order=order, rank=rank))
        per_core.append(passes)

    # cross-core per-subwindow max degree -> shared group schema per pass
    schema = []   # per pass: list of (sw0, nsub, F)
    for k in range(NCHUNK):
        fsub = np.zeros(COLS, np.int64)
        for c in range(NC):
            cnt = per_core[c][k]["cnt"]
            fsub = np.maximum(fsub, cnt[0:SHARD:128])
        groups = []
        sw = 0
        while sw < COLS:
            f_g = int(fsub[sw])
            if f_g == 0:
                groups.append((sw, COLS - sw, 0))
                break
            nsub = 1
            while (sw + nsub < COLS and nsub < MAXSUB
                   and (nsub + 1) * int(fsub[sw]) <= MAXSLOT):
                nsub += 1
            groups.append((sw, nsub, f_g))
            sw += nsub
        schema.append(groups)

    cores = []
    for c in range(NC):
        gidx, qidx, cidx = [], [], []
        for k in range(NCHUNK):
            P = per_core[c][k]
            cols_k = []
            for (sw0, nsub, f_g) in schema[k]:
                if f_g == 0:
                    continue
                ni = 128 * nsub * f_g
                j = np.arange(ni)
                p = j % 128
                b = j // 128
                r = (sw0 + b // f_g) * 128 + p
                f = b % f_g
                idxf = np.full(ni, RCH, np.int64)      # chunk dummy row
                valid = f < P["cnt"][r]
                ei = P["starts"][r] + f
                idxf[valid] = P["sk"][ei[valid]]
                cols_k.append(_wrap_idx(idxf))
            gidx.append(np.concatenate(cols_k, axis=1) if cols_k
                        else np.zeros((128, 0), np.int16))
            qidx.append(_wrap_idx(P["order"]))
            jc = np.arange(SHARD)
            nloc = (jc % 128) * COLS + jc // 128
            rr = P["rank"][nloc]
            cidx.append(_wrap_idx((rr % 128) * COLS + rr // 128))
        cores.append(dict(
            gidx=np.concatenate(gidx, axis=1),
            qidx=np.stack(qidx), cidx=np.stack(cidx)))
    return schema, cores


def _perm_h_src(h):
    hp = np.zeros((NPAD, D + 1), np.float32)
    r = np.arange(NPAD)
    rin = r % CROWS
    n = (r // CROWS) * RCH + rin
    real = (rin < RCH) & (n < N)
    hp[real, :D] = h[n[real]]
    hp[real, D] = 1.0
    return hp


def _local_h_dst(h, c):
    hp = np.zeros((SHARD, D + 1), np.float32)
    n_lo = c * SHARD
    nn = min(SHARD, N - n_lo)
    hp[:nn, :D] = h[n_lo:n_lo + nn]
    hp[:nn, D] = 1.0
    return hp


def _blockT(x):   # [12544, 65] -> [98, 65, 128]
    return np.ascontiguousarray(x.reshape(COLS, 128, D + 1).transpose(0, 2, 1))


def _mkM(W, wat):
    M = np.zeros((D + 1, 66), np.float32)
    M[:D, :D] = W.T
    M[D, D] = 1.0
    M[:D, 65] = W.T @ wat
    return M


# ---------------------------------------------------------------- device
def _build_program(schema, gcols, repeat):
    from concourse import bass, bacc, mybir, tile
    from concourse.library_config import mlp as mlp_lib
    f32, f16, i16 = mybir.dt.float32, mybir.dt.float16, mybir.dt.int16
    FN = mybir.ActivationFunctionType
    OP = mybir.AluOpType

    SLOT = max([MAXSLOT] + [n * f for p in schema for (_, n, f) in p])
    nc = bacc.Bacc("TRN2", target_bir_lowering=False, debug=False,
                   num_devices=NC, num_swdge_queues=4)
    hsT_e = nc.dram_tensor("hsT", [COLS, D + 1, 128], f32,
                           kind="ExternalInput")
    hdT_e = nc.dram_tensor("hdT", [COLS, D + 1, 128], f32,
                           kind="ExternalInput")
    mm_e = nc.dram_tensor("mm", [D + 1, 66], f32, kind="ExternalInput")
    gidx_e = nc.dram_tensor("gidx", [128, gcols], i16, kind="ExternalInput")
    qidx_e = nc.dram_tensor("qidx", [NCHUNK, 128, SHARD // 16], i16,
                            kind="ExternalInput")
    cidx_e = nc.dram_tensor("cidx", [NCHUNK, 128, SHARD // 16], i16,
                            kind="ExternalInput")
    res_e = nc.dram_tensor("res", [SHARD, D], f32, kind="ExternalOutput")

    with tile.TileContext(nc) as tc:
        with tc.tile_pool(name="cst", bufs=1) as cp, \
             tc.tile_pool(name="mn", bufs=1) as sp, \
             tc.tile_pool(name="dr", bufs=1, space="DRAM") as dp:
            bp = tc.alloc_tile_pool(name="bld", bufs=3)
            pp = tc.alloc_tile_pool(name="ps", bufs=2, space="PSUM")
            nc.gpsimd.load_library(mlp_lib)
            mm = cp.tile([D + 1, 66], f32)
            nc.sync.dma_start(out=mm[:], in_=mm_e[:])

            tsrc_sh = dp.tile([SHARD, ELEM], f16)
            tdst = dp.tile([SHARD, D], f16)
            qtab = dp.tile([SHARD, ELEM], f16)
            for b in range(COLS):
                hs = bp.tile([D + 1, 128], f32, tag="hs")
                nc.sync.dma_start(out=hs[:], in_=hsT_e[b])
                ps = pp.tile([128, 66], f32, space="PSUM", tag="ps")
                nc.tensor.matmul(out=ps[:], lhsT=hs[:], rhs=mm[:],
                                 start=True, stop=True)
                t16 = bp.tile([128, ELEM], f16, tag="t16")
                nc.vector.tensor_copy(t16[:, 0:66], ps[:])
                nc.sync.dma_start(out=tsrc_sh[b * 128:(b + 1) * 128, :],
                                  in_=t16[:])
                hd = bp.tile([D + 1, 128], f32, tag="hd")
                nc.sync.dma_start(out=hd[:], in_=hdT_e[b])
                ps2 = pp.tile([128, 66], f32, space="PSUM", tag="ps2")
                nc.tensor.matmul(out=ps2[:], lhsT=hd[:], rhs=mm[:],
                                 start=True, stop=True)
                td = bp.tile([128, D], f16, tag="td")
                nc.vector.tensor_copy(td[:], ps2[:, 0:64])
                nc.sync.dma_start(out=tdst[b * 128:(b + 1) * 128, :],
                                  in_=td[:])
                q16 = bp.tile([128, ELEM], f16, tag="q16")
                nc.scalar.activation(out=q16[:, 0:1], in_=ps2[:, 65:66],
                                     func=FN.Copy)
                nc.sync.dma_start(out=qtab[b * 128:(b + 1) * 128, :],
                                  in_=q16[:])

            tsrc = dp.tile([NPAD, ELEM], f16)
            nc.gpsimd.collective_compute(
                "AllGather", OP.bypass, replica_groups=[list(range(NC))],
                ins=[tsrc_sh.opt()], outs=[tsrc.opt()])

            # qd per pass in rank layout: [128, NCHUNK*COLS] f32
            qd = cp.tile([128, NCHUNK * COLS], f32)
            QSPL = [24, 24, 25, 25]     # col-blocks per sub-gather
            for k in range(NCHUNK):
                qi = bp.tile([128, SHARD // 16], i16, tag="qi")
                nc.sync.dma_start(out=qi[:], in_=qidx_e[k])
                c0 = 0
                for h, nb in enumerate(QSPL):
                    nih = nb * 128
                    qrows = sp.tile([128, SLOT * ELEM], f16, tag="pay",
                                    bufs=3)
                    qv = qrows[:, :nih].rearrange("p (a b) -> p a b", b=ELEM)
                    nc.gpsimd.dma_gather(
                        qv, qtab[:], qi[:, c0 * 8:(c0 + nb) * 8],
                        nih, nih, ELEM, single_packet=False,
                        queue_num=h % 4)
                    nc.vector.tensor_copy(
                        qd[:, k * COLS + c0:k * COLS + c0 + nb],
                        qv[:, :, 0:1].squeeze(2))
                    c0 += nb
            ci = []
            for k in range(NCHUNK):
                t = cp.tile([128, SHARD // 16], i16, tag=f"ci{k}")
                nc.sync.dma_start(out=t[:], in_=cidx_e[k])
                ci.append(t)
            bp.release()

            parts = [dp.tile([128, COLS, ELEM], f16, name=f"part{k}")
                     for k in range(NCHUNK)]

            rep = tc.For_i(0, repeat, 1) if repeat > 1 else None
            if rep is not None:
                rep.__enter__()
            off = 0
            qrr = [0]
            acc16 = sp.tile([128, COLS, 66], f16, tag="acc16")
            for k in range(NCHUNK):
                for (sw0, nsub, f_g) in schema[k]:
                    if f_g == 0:
                        nz = nsub
                        zc = sw0
                        while nz > 0:
                            zn = min(nz, MAXSUB)
                            zt = sp.tile([128, MAXSUB, ELEM], f16, tag="zt",
                                         bufs=2)
                            nc.vector.memset(zt[:, :zn, :], 0.0)
                            nc.sync.dma_start(
                                out=parts[k][:, zc:zc + zn, :],
                                in_=zt[:, :zn, :])
                            zc += zn
                            nz -= zn
                        continue
                    nig = 128 * nsub * f_g
                    ncol = nig // 16
                    it = sp.tile([128, SLOT * 8], i16, tag="it", bufs=2)
                    nc.sync.dma_start(out=it[:, :ncol],
                                      in_=gidx_e[:, off:off + ncol])
                    off += ncol
                    pay = sp.tile([128, SLOT * ELEM], f16, tag="pay",
                                  bufs=3)
                    pay4 = pay[:, :nsub * f_g * ELEM].rearrange(
                        "p (s f e) -> p s f e", s=nsub, e=ELEM)
                    nc.gpsimd.dma_gather(
                        pay[:, :nsub * f_g * ELEM].rearrange(
                            "p (a b) -> p a b", b=ELEM),
                        tsrc[k * CROWS:(k + 1) * CROWS, :],
                        it[:, :ncol], nig, nig, ELEM, single_packet=False,
                        queue_num=qrr[0] % 4)
                    qrr[0] += 1
                    # per-edge weight: w = exp(tanh(qd - qs))
                    dif = sp.tile([128, SLOT], f32, tag="dif", bufs=2)
                    difv = dif[:, :nsub * f_g].rearrange(
                        "p (s f) -> p s f", s=nsub)
                    nc.vector.tensor_tensor(
                        difv,
                        qd[:, k * COLS + sw0:k * COLS + sw0 + nsub]
                        .unsqueeze(2).broadcast_to((128, nsub, f_g)),
                        pay4[:, :, :, 65], op=OP.subtract)
                    th = sp.tile([128, SLOT], f32, tag="th", bufs=2)
                    nc.scalar.activation(out=th[:, :nsub * f_g],
                                         in_=dif[:, :nsub * f_g],
                                         func=FN.Tanh)
                    w16 = sp.tile([128, SLOT], f16, tag="w16", bufs=2)
                    nc.scalar.activation(out=w16[:, :nsub * f_g],
                                         in_=th[:, :nsub * f_g], func=FN.Exp)
                    # weighted rows in place: pay[:,:,:,0:65] *= w
                    payT = pay4[:, :, :, 0:65].transpose([0, 1, 3, 2])
                    nc.vector.tensor_tensor(
                        payT, payT,
                        w16[:, :nsub * f_g].rearrange("p (s f) -> p s f",
                                                      s=nsub)
                        .unsqueeze(2).broadcast_to((128, nsub, 65, f_g)),
                        op=OP.mult)
                    acc = sp.tile([128, MAXSUB * 65], f32, tag="acc", bufs=2)
                    nc.vector.tensor_reduce(
                        out=acc[:, :nsub * 65].rearrange(
                            "p (s e) -> p s e", s=nsub),
                        in_=payT, axis=mybir.AxisListType.X, op=OP.add)
                    pout = sp.tile([128, MAXSUB, ELEM], f16, tag="pout",
                                   bufs=2)
                    nc.scalar.activation(
                        out=pout[:, :nsub, 0:65],
                        in_=acc[:, :nsub * 65].rearrange(
                            "p (s e) -> p s e", s=nsub), func=FN.Copy)
                    nc.sync.dma_start(out=parts[k][:, sw0:sw0 + nsub, :],
                                      in_=pout[:, :nsub, :])
                # combine pass k's partials (overlaps next pass's gathers)
                pt = sp.tile([128, COLS * ELEM], f16, tag="pt", bufs=1)
                ptv = pt[:].rearrange("p (a b) -> p a b", b=ELEM)
                nc.gpsimd.dma_gather(
                    ptv, parts[k][:].rearrange("p g e -> (p g) e"),
                    ci[k][:], SHARD, SHARD, ELEM, single_packet=False,
                    queue_num=k % 4)
                if k == 0:
                    nc.vector.tensor_copy(acc16[:], ptv[:, :, 0:66])
                else:
                    nc.vector.tensor_tensor(acc16[:], acc16[:],
                                            ptv[:, :, 0:66], op=OP.add)

            # ---- epilogue (canonical layout: node = p*98 + col)
            pd = sp.tile([128, COLS, D], f16, tag="pd")
            nc.sync.dma_start(
                out=pd[:], in_=tdst[:].rearrange("(p c) e -> p c e", p=128))
            sw_ = acc16[:, :, 64:65].squeeze(2)
            z = sp.tile([128, COLS], f32, tag="z")
            nc.vector.tensor_scalar(out=z[:], in0=sw_, scalar1=0.0,
                                    scalar2=None, op0=OP.is_equal)
            den = sp.tile([128, COLS], f32, tag="den")
            nc.vector.tensor_tensor(den[:], sw_, z[:], op=OP.add)
            rec = sp.tile([128, COLS], f32, tag="rec")
            nc.vector.reciprocal(rec[:], den[:])
            nzm = sp.tile([128, COLS], f32, tag="nzm")
            nc.vector.tensor_scalar(out=nzm[:], in0=z[:], scalar1=-1.0,
                                    scalar2=1.0, op0=OP.mult, op1=OP.add)
            mean = sp.tile([128, COLS, D], f16, tag="e16", bufs=2)
            nc.vector.tensor_tensor(
                mean[:], acc16[:, :, 0:64],
                rec[:].unsqueeze(2).broadcast_to((128, COLS, D)), op=OP.mult)
            df = sp.tile([128, COLS, D], f16, tag="df")
            nc.vector.tensor_tensor(df[:], pd[:], mean[:], op=OP.subtract)
            nc.vector.tensor_tensor(
                df[:], df[:],
                nzm[:].unsqueeze(2).broadcast_to((128, COLS, D)), op=OP.mult)
            ng = sp.tile([128, COLS, D], f16, tag="e16", bufs=2)
            nc.vector.tensor_scalar(out=ng[:], in0=df[:], scalar1=0.0,
                                    scalar2=None, op0=OP.min)
            ex = sp.tile([128, COLS, D], f16, tag="e16", bufs=2)
            nc.scalar.activation(out=ex[:], in_=ng[:], func=FN.Exp)
            nc.vector.tensor_scalar(out=df[:], in0=df[:], scalar1=0.0,
                                    scalar2=None, op0=OP.max)
            resf = sp.tile([128, COLS, D], f32, tag="resf")
            nc.vector.scalar_tensor_tensor(
                out=resf[:], in0=ex[:], scalar=-1.0, in1=df[:],
                op0=OP.add, op1=OP.add)
            nc.sync.dma_start(
                out=res_e[:].rearrange("(p c) e -> p c e", p=128),
                in_=resf[:])
            if rep is not None:
                rep.__exit__(None, None, None)
            pp.release()
    nc.compile()
    return nc


_CACHE = {}


def _get_program(schema, gcols, repeat):
    key = (tuple(tuple(g) for p in schema for g in p), gcols, repeat)
    if key not in _CACHE:
        _CACHE[key] = _build_program(schema, gcols, repeat)
    return _CACHE[key]


def kernel(h_src, h_dst, W_fc, w_attn, src, dst, _main_repeat=MAIN_REPEAT):
    from concourse.bass_utils import run_bass_kernel_spmd

    h_src = np.ascontiguousarray(np.asarray(h_src, np.float32))
    h_dst = np.ascontiguousarray(np.asarray(h_dst, np.float32))
    W_fc = np.ascontiguousarray(np.asarray(W_fc, np.float32))
    w_attn = np.ascontiguousarray(np.asarray(w_attn, np.float32)).reshape(D)
    schema, cores = _prep(src, dst)
    gcols = cores[0]["gidx"].shape[1]

    hsp = _perm_h_src(h_src)
    M = _mkM(W_fc, w_attn)
    in_maps = []
    for c in range(NC):
        in_maps.append({
            "hsT": _blockT(hsp[c * SHARD:(c + 1) * SHARD]),
            "hdT": _blockT(_local_h_dst(h_dst, c)),
            "mm": M,
            "gidx": cores[c]["gidx"],
            "qidx": cores[c]["qidx"],
            "cidx": cores[c]["cidx"],
        })
    nc = _get_program(schema, gcols, _main_repeat)
    res = run_bass_kernel_spmd(nc, in_maps, list(range(NC)))

    out = np.zeros((N, D), np.float32)
    for c in range(NC):
        nn = min(SHARD, N - c * SHARD)
        out[c * SHARD:c * SHARD + nn] = res.results[c]["res"][:nn]
    return out


# ---------------------------------------------------------------- local sim
def simulate(h_src, h_dst, W_fc, w_attn, src, dst):
    """Numpy mirror of the device program (incl. fp16 quantization)."""
    h_src = np.asarray(h_src, np.float32)
    h_dst = np.asarray(h_dst, np.float32)
    W_fc = np.asarray(W_fc, np.float32)
    w_attn = np.asarray(w_attn, np.float32).reshape(D)
    schema, cores = _prep(src, dst)
    M = _mkM(W_fc, w_attn)
    hsp = _perm_h_src(h_src)
    tab16 = (hsp @ M).astype(np.float16)         # [NPAD, 66]
    out = np.zeros((N, D), np.float32)
    for c in range(NC):
        hd = _local_h_dst(h_dst, c)
        pdq = hd @ M                              # [SHARD, 66] f32
        qtab16 = pdq[:, 65].astype(np.float16)
        parts = []
        src64 = np.asarray(src, np.int64)
        dst64 = np.asarray(dst, np.int64)
        P = cores[c]
        # recompute per-pass structures (same as _prep)
        n_lo = c * SHARD
        e_lo = np.searchsorted(dst64, n_lo)
        e_hi = np.searchsorted(dst64, min(n_lo + SHARD, N))
        s_ = src64[e_lo:e_hi]
        d_ = dst64[e_lo:e_hi] - n_lo
        for k in range(NCHUNK):
            part = np.zeros((SHARD, 66), np.float16)   # row = p*98 + G
            m = (s_ // RCH) == k
            deg = np.bincount(d_[m], minlength=SHARD)
            order = np.argsort(-deg, kind="stable")
            qd_rank = qtab16[order].astype(np.float32)   # [rank]
            # decode gidx arrays back? simpler: recompute idxf same way
            sk = (s_[m] % RCH).astype(np.int64)
            rank = np.empty(SHARD, np.int64)
            rank[order] = np.arange(SHARD)
            eo = np.argsort(rank[d_[m]], kind="stable")
            sk = sk[eo]
            cnt = deg[order]
            starts = np.concatenate([[0], np.cumsum(cnt)])
            for (sw0, nsub, f_g) in schema[k]:
                if f_g == 0:
                    continue
                ni = 128 * nsub * f_g
                j = np.arange(ni)
                p = j % 128
                b = j // 128
                r = (sw0 + b // f_g) * 128 + p
                f = b % f_g
                idxf = np.full(ni, RCH, np.int64)
                valid = f < cnt[r]
                idxf[valid] = sk[(starts[r] + f)[valid]]
                rows = tab16[k * CROWS + idxf]            # [ni, 66]
                qs = rows[:, 65].astype(np.float32)
                dif = qd_rank[r] - qs
                w16 = np.exp(np.tanh(dif)).astype(np.float16)
                wp = (rows[:, 0:65] * w16[:, None]).astype(np.float16)
                acc = wp.astype(np.float32).reshape(nsub, f_g, 128, 65) \
                    .sum(axis=1)                           # [nsub? ...]
                # careful: j order is (b=(sub,f), p): reshape [(nsub f) 128]
                part_rows = acc.astype(np.float16)         # [nsub, 128, 65]
                for s2 in range(nsub):
                    G = sw0 + s2
                    part[np.arange(128) * COLS + G, 0:65] = part_rows[s2]
            parts.append(part)
        # combine in canonical layout
        acc16 = np.zeros((SHARD, 66), np.float16)
        for k in range(NCHUNK):
            m = (s_ // RCH) == k
            deg = np.bincount(d_[m], minlength=SHARD)
            rank = np.empty(SHARD, np.int64)
            rank[np.argsort(-deg, kind="stable")] = np.arange(SHARD)
            nloc = np.arange(SHARD)
            rr = rank[nloc]
            rowid = (rr % 128) * COLS + rr // 128
            acc16 = (acc16 + parts[k][rowid]).astype(np.float16)
        swv = acc16[:, 64].astype(np.float32)
        z = (swv == 0.0).astype(np.float32)
        rec = 1.0 / (swv + z)
        nzm = 1.0 - z
        mean = (acc16[:, 0:64].astype(np.float32)
                * rec[:, None]).astype(np.float16).astype(np.float32)
        pd16 = pdq[:, 0:64].astype(np.float16).astype(np.float32)
        df = ((pd16 - mean).astype(np.float16).astype(np.float32)
              * nzm[:, None]).astype(np.float16).astype(np.float32)
        resv = np.where(df > 0, df, np.expm1(np.minimum(df, 0)))
        nn = min(SHARD, N - c * SHARD)
        out[c * SHARD:c * SHARD + nn] = resv[:nn]
    return out


if __name__ == "__main__":
    d = np.load("/root/problem/refdata.npz")
    o = kernel(d["h_src"], d["h_dst"], d["W_fc"], d["w_attn"],
               d["src"], d["dst"])
    exp = d["expected"]
    rel = np.linalg.norm(o - exp) / np.linalg.norm(exp)
    print(f"rel_l2 = {rel:.3e}")


# revision 5
# speedup vs baseline: 2.1638x; 1.5579x over previous
"""Trainium2 Bass kernel for nn_DiffAttention — node-major 4-pass dma_gather.

Math (edge i: src s -> dst n, per-dst softmax over incoming edges):
  p = h @ W_fc.T ; q = p @ w_attn ; w_i = exp(tanh(q_dst[n] - q_src[s]))
  out[n] = elu(p_dst[n] - (sum_i w_i p_src[s_i]) / (sum_i w_i))
(e = tanh(..) in [-1,1] so softmax max-subtraction is unnecessary.)

Device strategy (8 cores, SPMD, dst-sharded 12544 nodes/core):
  - fp16 src table [100352, 128]: rows [p(64)|one|q|junk], node id permuted
    into 4 chunks of 25088 rows (25087 real + 1 zero dummy) so every
    dma_gather idx fits in int16. Built sharded on PE, AllGather'd.
  - 4 passes per core: pass k covers edges with src in chunk k. Nodes are
    re-sorted by pass-degree; groups of <=8 subwindows x 128 nodes share a
    uniform per-node slot count F. One dma_gather per group fetches all
    edge rows node-major: slot j -> partition j%128, block j//128 = (sub,f).
    Per-edge w on ACT (qd is per-partition!), weighted rows by in-place DVE
    mult, per-node sums by DVE reduce along f. Partials [swp|sw] -> fp16
    tables in pass order.
  - Combine: per pass one dma_gather re-orders partials to canonical node
    layout (p=n//98, col=n%98); sum, then batched epilogue
    elu(p_dst - swp/sw) with zero-degree masking; one plain DMA out.
Host does only index prep (degree sorts, idx arrays, permuted h copies).
"""
import sys
sys.path.insert(0, "/opt/trn_rl_repo")
import numpy as np

N = 100000
D = 64
NC = 8
SHARD = 12544            # nodes per core = 128 * 98
COLS = 98
RCH = 25087              # real nodes per chunk
CROWS = 25088            # table rows per chunk (last row zero dummy)
NPAD = CROWS * 4         # 100352
NCHUNK = 4
ELEM = 128               # fp16 elems per src-table row (256B)
MAXSLOT = 32             # max nsub*F per gather group (pay tile 8KB/part)
MAXSUB = 8
MAIN_REPEAT = 1


# ---------------------------------------------------------------- host prep
def _wrap_idx(flat):
    """[n] int -> [128, n//16] int16, idx j at [16s + j%16, j//16] stripes
    replicated (HW SWDGE reads stripe 16:32; interp reads 0:16)."""
    w = flat.reshape(-1, 16).T
    return np.ascontiguousarray(np.tile(w, (8, 1)).astype(np.int16))


def _prep(src, dst):
    src = np.asarray(src, np.int64)
    dst = np.asarray(dst, np.int64)
    if np.any(np.diff(dst) < 0):
        o = np.argsort(dst, kind="stable")
        src, dst = src[o], dst[o]
    per_core = []
    for c in range(NC):
        n_lo = c * SHARD
        e_lo = np.searchsorted(dst, n_lo)
        e_hi = np.searchsorted(dst, min(n_lo + SHARD, N))
        s = src[e_lo:e_hi]
        d = dst[e_lo:e_hi] - n_lo
        passes = []
        for k in range(NCHUNK):
            m = (s // RCH) == k
            sk = (s[m] % RCH).astype(np.int64)
            dk = d[m]
            deg = np.bincount(dk, minlength=SHARD)
            order = np.argsort(-deg, kind="stable")
            rank = np.empty(SHARD, np.int64)
            rank[order] = np.arange(SHARD)
            eo = np.argsort(rank[dk], kind="stable")
            sk = sk[eo]
            cnt = deg[order]
            starts = np.concatenate([[0], np.cumsum(cnt)])
            passes.append(dict(cnt=cnt, starts=starts, sk=sk,
                               order=order, rank=rank))
        per_core.append(passes)

    # cross-core per-subwindow max degree -> shared group schema per pass
    schema = []   # per pass: list of (sw0, nsub, F)
    for k in range(NCHUNK):
        fsub = np.zeros(COLS, np.int64)
        for c in range(NC):
            cnt = per_core[c][k]["cnt"]
            fsub = np.maximum(fsub, cnt[0:SHARD:128])
        groups = []
        sw = 0
        while sw < COLS:
            f_g = int(fsub[sw])
            if f_g == 0:
                groups.append((sw, COLS - sw, 0))
                break
            nsub = 1
            while (sw + nsub < COLS and nsub < MAXSUB
                   and (nsub + 1) * int(fsub[sw]) <= MAXSLOT):
                nsub += 1
            groups.append((sw, nsub, f_g))
            sw += nsub
        schema.append(groups)

    cores = []
    for c in range(NC):
        gidx, qidx, cidx = [], [], []
        for k in range(NCHUNK):
            P = per_core[c][k]
            cols_k = []
            for (sw0, nsub, f_g) in schema[k]:
                if f_g == 0:
                    continue
                ni = 128 * nsub * f_g
                j = np.arange(ni)
                p = j % 128
                b = j // 128
                r = (sw0 + b // f_g) * 128 + p
                f = b % f_g
                idxf = np.full(ni, RCH, np.int64)      # chunk dummy row
                valid = f < P["cnt"][r]
                ei = P["starts"][r] + f
                idxf[valid] = P["sk"][ei[valid]]
                cols_k.append(_wrap_idx(idxf))
            gidx.append(np.concatenate(cols_k, axis=1) if cols_k
                        else np.zeros((128, 0), np.int16))
            qidx.append(_wrap_idx(P["order"]))
            jc = np.arange(SHARD)
            nloc = (jc % 128) * COLS + jc // 128
            rr = P["rank"][nloc]
            cidx.append(_wrap_idx((rr % 128) * COLS + rr // 128))
        cores.append(dict(
            gidx=np.concatenate(gidx, axis=1),
            qidx=np.stack(qidx), cidx=np.stack(cidx)))
    return schema, cores


def _perm_h_src(h):
    hp = np.zeros((NPAD, D + 1), np.float32)
    r = np.arange(NPAD)
    rin = r % CROWS
    n = (r // CROWS) * RCH + rin
    real = (rin < RCH) & (n < N)
    hp[real, :D] = h[n[real]]
    hp[real, D] = 1.0
    return hp


def _local_h_dst(h, c):
    hp = np.zeros((SHARD, D + 1), np.float32)
    n_lo = c * SHARD
    nn = min(SHARD, N - n_lo)
    hp[:nn, :D] = h[n_lo:n_lo + nn]
    hp[:nn, D] = 1.0
    return hp


def _blockT(x):   # [12544, 65] -> [98, 65, 128]
    return np.ascontiguousarray(x.reshape(COLS, 128, D + 1).transpose(0, 2, 1))


def _mkM(W, wat):
    M = np.zeros((D + 1, 66), np.float32)
    M[:D, :D] = W.T
    M[D, D] = 1.0
    M[:D, 65] = W.T @ wat
    return M


# ---------------------------------------------------------------- device
def _build_program(schema, gcols, repeat):
    from concourse import bass, bacc, mybir, tile
    from concourse.library_config import mlp as mlp_lib
    f32, f16, i16 = mybir.dt.float32, mybir.dt.float16, mybir.dt.int16
    FN = mybir.ActivationFunctionType
    OP = mybir.AluOpType

    SLOT = max([MAXSLOT] + [n * f for p in schema for (_, n, f) in p])
    nc = bacc.Bacc("TRN2", target_bir_lowering=False, debug=False,
                   num_devices=NC, num_swdge_queues=4)
    hsT_e = nc.dram_tensor("hsT", [COLS, D + 1, 128], f32,
                           kind="ExternalInput")
    hdT_e = nc.dram_tensor("hdT", [COLS, D + 1, 128], f32,
                           kind="ExternalInput")
    mm_e = nc.dram_tensor("mm", [D + 1, 66], f32, kind="ExternalInput")
    gidx_e = nc.dram_tensor("gidx", [128, gcols], i16, kind="ExternalInput")
    qidx_e = nc.dram_tensor("qidx", [NCHUNK, 128, SHARD // 16], i16,
                            kind="ExternalInput")
    cidx_e = nc.dram_tensor("cidx", [NCHUNK, 128, SHARD // 16], i16,
                            kind="ExternalInput")
    res_e = nc.dram_tensor("res", [SHARD, D], f32, kind="ExternalOutput")

    with tile.TileContext(nc) as tc:
        with tc.tile_pool(name="cst", bufs=1) as cp, \
             tc.tile_pool(name="mn", bufs=1) as sp, \
             tc.tile_pool(name="dr", bufs=1, space="DRAM") as dp:
            bp = tc.alloc_tile_pool(name="bld", bufs=3)
            pp = tc.alloc_tile_pool(name="ps", bufs=2, space="PSUM")
            nc.gpsimd.load_library(mlp_lib)
            mm = cp.tile([D + 1, 66], f32)
            nc.sync.dma_start(out=mm[:], in_=mm_e[:])

            tsrc_sh = dp.tile([SHARD, ELEM], f16)
            tdst = dp.tile([SHARD, D], f16)
            qtab = dp.tile([SHARD, ELEM], f16)
            for b in range(COLS):
                hs = bp.tile([D + 1, 128], f32, tag="hs")
                nc.sync.dma_start(out=hs[:], in_=hsT_e[b])
                ps = pp.tile([128, 66], f32, space="PSUM", tag="ps")
                nc.tensor.matmul(out=ps[:], lhsT=hs[:], rhs=mm[:],
                                 start=True, stop=True)
                t16 = bp.tile([128, ELEM], f16, tag="t16")
                nc.vector.tensor_copy(t16[:, 0:66], ps[:])
                nc.sync.dma_start(out=tsrc_sh[b * 128:(b + 1) * 128, :],
                                  in_=t16[:])
                hd = bp.tile([D + 1, 128], f32, tag="hd")
                nc.sync.dma_start(out=hd[:], in_=hdT_e[b])
                ps2 = pp.tile([128, 66], f32, space="PSUM", tag="ps2")
                nc.tensor.matmul(out=ps2[:], lhsT=hd[:], rhs=mm[:],
                                 start=True, stop=True)
                td = bp.tile([128, D], f16, tag="td")
                nc.vector.tensor_copy(td[:], ps2[:, 0:64])
                nc.sync.dma_start(out=tdst[b * 128:(b + 1) * 128, :],
                                  in_=td[:])
                q16 = bp.tile([128, ELEM], f16, tag="q16")
                nc.scalar.activation(out=q16[:, 0:1], in_=ps2[:, 65:66],
                                     func=FN.Copy)
                nc.sync.dma_start(out=qtab[b * 128:(b + 1) * 128, :],
                                  in_=q16[:])

            tsrc = dp.tile([NPAD, ELEM], f16)
            nc.gpsimd.collective_compute(
                "AllGather", OP.bypass, replica_groups=[list(range(NC))],
                ins=[tsrc_sh.opt()], outs=[tsrc.opt()])

            # qd per pass in rank layout: [128, NCHUNK*COLS] f32
            qd = cp.tile([128, NCHUNK * COLS], f32)
            QSPL = [24, 24, 25, 25]     # col-blocks per sub-gather
            for k in range(NCHUNK):
                qi = bp.tile([128, SHARD // 16], i16, tag="qi")
                nc.sync.dma_start(out=qi[:], in_=qidx_e[k])
                c0 = 0
                for h, nb in enumerate(QSPL):
                    nih = nb * 128
                    qrows = sp.tile([128, SLOT * ELEM], f16, tag="pay",
                                    bufs=5)
                    qv = qrows[:, :nih].rearrange("p (a b) -> p a b", b=ELEM)
                    nc.gpsimd.dma_gather(
                        qv, qtab[:], qi[:, c0 * 8:(c0 + nb) * 8],
                        nih, nih, ELEM, single_packet=False,
                        queue_num=h % 4)
                    nc.vector.tensor_copy(
                        qd[:, k * COLS + c0:k * COLS + c0 + nb],
                        qv[:, :, 0:1].squeeze(2))
                    c0 += nb
            ci = []
            for k in range(NCHUNK):
                t = cp.tile([128, SHARD // 16], i16, tag=f"ci{k}")
                nc.sync.dma_start(out=t[:], in_=cidx_e[k])
                ci.append(t)
            bp.release()

            parts = [dp.tile([128, COLS, ELEM], f16, name=f"part{k}")
                     for k in range(NCHUNK)]

            rep = tc.For_i(0, repeat, 1) if repeat > 1 else None
            if rep is not None:
                rep.__enter__()
            off = 0
            qrr = [0]
            acc16 = sp.tile([128, COLS, 66], f16, tag="acc16")
            for k in range(NCHUNK):
                for (sw0, nsub, f_g) in schema[k]:
                    if f_g == 0:
                        nz = nsub
                        zc = sw0
                        while nz > 0:
                            zn = min(nz, MAXSUB)
                            zt = sp.tile([128, MAXSUB, ELEM], f16, tag="zt",
                                         bufs=2)
                            nc.vector.memset(zt[:, :zn, :], 0.0)
                            nc.sync.dma_start(
                                out=parts[k][:, zc:zc + zn, :],
                                in_=zt[:, :zn, :])
                            zc += zn
                            nz -= zn
                        continue
                    nig = 128 * nsub * f_g
                    ncol = nig // 16
                    it = sp.tile([128, SLOT * 8], i16, tag="it", bufs=5)
                    nc.sync.dma_start(out=it[:, :ncol],
                                      in_=gidx_e[:, off:off + ncol])
                    off += ncol
                    pay = sp.tile([128, SLOT * ELEM], f16, tag="pay",
                                  bufs=5)
                    pay4 = pay[:, :nsub * f_g * ELEM].rearrange(
                        "p (s f e) -> p s f e", s=nsub, e=ELEM)
                    nc.gpsimd.dma_gather(
                        pay[:, :nsub * f_g * ELEM].rearrange(
                            "p (a b) -> p a b", b=ELEM),
                        tsrc[k * CROWS:(k + 1) * CROWS, :],
                        it[:, :ncol], nig, nig, ELEM, single_packet=False,
                        queue_num=qrr[0] % 4)
                    qrr[0] += 1
                    # per-edge weight: w = exp(tanh(qd - qs))
                    dif = sp.tile([128, SLOT], f32, tag="dif", bufs=3)
                    difv = dif[:, :nsub * f_g].rearrange(
                        "p (s f) -> p s f", s=nsub)
                    nc.vector.tensor_tensor(
                        difv,
                        qd[:, k * COLS + sw0:k * COLS + sw0 + nsub]
                        .unsqueeze(2).broadcast_to((128, nsub, f_g)),
                        pay4[:, :, :, 65], op=OP.subtract)
                    th = sp.tile([128, SLOT], f32, tag="th", bufs=3)
                    nc.scalar.activation(out=th[:, :nsub * f_g],
                                         in_=dif[:, :nsub * f_g],
                                         func=FN.Tanh)
                    w16 = sp.tile([128, SLOT], f16, tag="w16", bufs=3)
                    nc.scalar.activation(out=w16[:, :nsub * f_g],
                                         in_=th[:, :nsub * f_g], func=FN.Exp)
                    # weighted rows in place: pay[:,:,:,0:65] *= w
                    payT = pay4[:, :, :, 0:65].transpose([0, 1, 3, 2])
                    nc.vector.tensor_tensor(
                        payT, payT,
                        w16[:, :nsub * f_g].rearrange("p (s f) -> p s f",
                                                      s=nsub)
                        .unsqueeze(2).broadcast_to((128, nsub, 65, f_g)),
                        op=OP.mult)
                    acc = sp.tile([128, MAXSUB * 65], f32, tag="acc", bufs=3)
                    nc.vector.tensor_reduce(
                        out=acc[:, :nsub * 65].rearrange(
                            "p (s e) -> p s e", s=nsub),
                        in_=payT, axis=mybir.AxisListType.X, op=OP.add)
                    pout = sp.tile([128, MAXSUB, ELEM], f16, tag="pout",
                                   bufs=3)
                    nc.scalar.activation(
                        out=pout[:, :nsub, 0:65],
                        in_=acc[:, :nsub * 65].rearrange(
                            "p (s e) -> p s e", s=nsub), func=FN.Copy)
                    nc.sync.dma_start(out=parts[k][:, sw0:sw0 + nsub, :],
                                      in_=pout[:, :nsub, :])
                # combine pass k's partials (overlaps next pass's gathers)
                pt = sp.tile([128, COLS * ELEM], f16, tag="pt", bufs=1)
                ptv = pt[:].rearrange("p (a b) -> p a b", b=ELEM)
                nc.gpsimd.dma_gather(
                    ptv, parts[k][:].rearrange("p g e -> (p g) e"),
                    ci[k][:], SHARD, SHARD, ELEM, single_packet=False,
                    queue_num=k % 4)
                if k == 0:
                    nc.vector.tensor_copy(acc16[:], ptv[:, :, 0:66])
                else:
                    nc.vector.tensor_tensor(acc16[:], acc16[:],
                                            ptv[:, :, 0:66], op=OP.add)

            # ---- epilogue (canonical layout: node = p*98 + col)
            pd = sp.tile([128, COLS, D], f16, tag="pd")
            nc.sync.dma_start(
                out=pd[:], in_=tdst[:].rearrange("(p c) e -> p c e", p=128))
            sw_ = acc16[:, :, 64:65].squeeze(2)
            z = sp.tile([128, COLS], f32, tag="z")
            nc.vector.tensor_scalar(out=z[:], in0=sw_, scalar1=0.0,
                                    scalar2=None, op0=OP.is_equal)
            den = sp.tile([128, COLS], f32, tag="den")
            nc.vector.tensor_tensor(den[:], sw_, z[:], op=OP.add)
            rec = sp.tile([128, COLS], f32, tag="rec")
            nc.vector.reciprocal(rec[:], den[:])
            nzm = sp.tile([128, COLS], f32, tag="nzm")
            nc.vector.tensor_scalar(out=nzm[:], in0=z[:], scalar1=-1.0,
                                    scalar2=1.0, op0=OP.mult, op1=OP.add)
            mean = sp.tile([128, COLS, D], f16, tag="e16", bufs=2)
            nc.vector.tensor_tensor(
                mean[:], acc16[:, :, 0:64],
                rec[:].unsqueeze(2).broadcast_to((128, COLS, D)), op=OP.mult)
            df = sp.tile([128, COLS, D], f16, tag="df")
            nc.vector.tensor_tensor(df[:], pd[:], mean[:], op=OP.subtract)
            nc.vector.tensor_tensor(
                df[:], df[:],
                nzm[:].unsqueeze(2).broadcast_to((128, COLS, D)), op=OP.mult)
            ng = sp.tile([128, COLS, D], f16, tag="e16", bufs=2)
            nc.vector.tensor_scalar(out=ng[:], in0=df[:], scalar1=0.0,
                                    scalar2=None, op0=OP.min)
            ex = sp.tile([128, COLS, D], f16, tag="e16", bufs=2)
            nc.scalar.activation(out=ex[:], in_=ng[:], func=FN.Exp)
            nc.vector.tensor_scalar(out=df[:], in0=df[:], scalar1=0.0,
                                    scalar2=None, op0=OP.max)
            resf = sp.tile([128, COLS, D], f32, tag="resf")
            nc.vector.scalar_tensor_tensor(
                out=resf[:], in0=ex[:], scalar=-1.0, in1=df[:],
                op0=OP.add, op1=OP.add)
            nc.sync.dma_start(
                out=res_e[:].rearrange("(p c) e -> p c e", p=128),
                in_=resf[:])
            if rep is not None:
                rep.__exit__(None, None, None)
            pp.release()
    nc.compile()
    return nc


_CACHE = {}


def _get_program(schema, gcols, repeat):
    key = (tuple(tuple(g) for p in schema for g in p), gcols, repeat)
    if key not in _CACHE:
        _CACHE[key] = _build_program(schema, gcols, repeat)
    return _CACHE[key]


def kernel(h_src, h_dst, W_fc, w_attn, src, dst, _main_repeat=MAIN_REPEAT):
    from concourse.bass_utils import run_bass_kernel_spmd

    h_src = np.ascontiguousarray(np.asarray(h_src, np.float32))
    h_dst = np.ascontiguousarray(np.asarray(h_dst, np.float32))
    W_fc = np.ascontiguousarray(np.asarray(W_fc, np.float32))
    w_attn = np.ascontiguousarray(np.asarray(w_attn, np.float32)).reshape(D)
    schema, cores = _prep(src, dst)
    gcols = cores[0]["gidx"].shape[1]

    hsp = _perm_h_src(h_src)
    M = _mkM(W_fc, w_attn)
    in_maps = []
    for c in range(NC):
        in_maps.append({
            "hsT": _blockT(hsp[c * SHARD:(c + 1) * SHARD]),
            "hdT": _blockT(_local_h_dst(h_dst, c)),
            "mm": M,
            "gidx": cores[c]["gidx"],
            "qidx": cores[c]["qidx"],
            "cidx": cores[c]["cidx"],
        })
    nc = _get_program(schema, gcols, _main_repeat)
    res = run_bass_kernel_spmd(nc, in_maps, list(range(NC)))

    out = np.zeros((N, D), np.float32)
    for c in range(NC):
        nn = min(SHARD, N - c * SHARD)
        out[c * SHARD:c * SHARD + nn] = res.results[c]["res"][:nn]
    return out


# ---------------------------------------------------------------- local sim
def simulate(h_src, h_dst, W_fc, w_attn, src, dst):
    """Numpy mirror of the device program (incl. fp16 quantization)."""
    h_src = np.asarray(h_src, np.float32)
    h_dst = np.asarray(h_dst, np.float32)
    W_fc = np.asarray(W_fc, np.float32)
    w_attn = np.asarray(w_attn, np.float32).reshape(D)
    schema, cores = _prep(src, dst)
    M = _mkM(W_fc, w_attn)
    hsp = _perm_h_src(h_src)
    tab16 = (hsp @ M).astype(np.float16)         # [NPAD, 66]
    out = np.zeros((N, D), np.float32)
    for c in range(NC):
        hd = _local_h_dst(h_dst, c)
        pdq = hd @ M                              # [SHARD, 66] f32
        qtab16 = pdq[:, 65].astype(np.float16)
        parts = []
        src64 = np.asarray(src, np.int64)
        dst64 = np.asarray(dst, np.int64)
        P = cores[c]
        # recompute per-pass structures (same as _prep)
        n_lo = c * SHARD
        e_lo = np.searchsorted(dst64, n_lo)
        e_hi = np.searchsorted(dst64, min(n_lo + SHARD, N))
        s_ = src64[e_lo:e_hi]
        d_ = dst64[e_lo:e_hi] - n_lo
        for k in range(NCHUNK):
            part = np.zeros((SHARD, 66), np.float16)   # row = p*98 + G
            m = (s_ // RCH) == k
            deg = np.bincount(d_[m], minlength=SHARD)
            order = np.argsort(-deg, kind="stable")
            qd_rank = qtab16[order].astype(np.float32)   # [rank]
            # decode gidx arrays back? simpler: recompute idxf same way
            sk = (s_[m] % RCH).astype(np.int64)
            rank = np.empty(SHARD, np.int64)
            rank[order] = np.arange(SHARD)
            eo = np.argsort(rank[d_[m]], kind="stable")
            sk = sk[eo]
            cnt = deg[order]
            starts = np.concatenate([[0], np.cumsum(cnt)])
            for (sw0, nsub, f_g) in schema[k]:
                if f_g == 0:
                    continue
                ni = 128 * nsub * f_g
                j = np.arange(ni)
                p = j % 128
                b = j // 128
                r = (sw0 + b // f_g) * 128 + p
                f = b % f_g
                idxf = np.full(ni, RCH, np.int64)
                valid = f < cnt[r]
                idxf[valid] = sk[(starts[r] + f)[valid]]
                rows = tab16[k * CROWS + idxf]            # [ni, 66]
                qs = rows[:, 65].astype(np.float32)
                dif = qd_rank[r] - qs
                w16 = np.exp(np.tanh(dif)).astype(np.float16)
                wp = (rows[:, 0:65] * w16[:, None]).astype(np.float16)
                acc = wp.astype(np.float32).reshape(nsub, f_g, 128, 65) \
                    .sum(axis=1)                           # [nsub? ...]
                # careful: j order is (b=(sub,f), p): reshape [(nsub f) 128]
                part_rows = acc.astype(np.float16)         # [nsub, 128, 65]
                for s2 in range(nsub):
                    G = sw0 + s2
                    part[np.arange(128) * COLS + G, 0:65] = part_rows[s2]
            parts.append(part)
        # combine in canonical layout
        acc16 = np.zeros((SHARD, 66), np.float16)
        for k in range(NCHUNK):
            m = (s_ // RCH) == k
            deg = np.bincount(d_[m], minlength=SHARD)
            rank = np.empty(SHARD, np.int64)
            rank[np.argsort(-deg, kind="stable")] = np.arange(SHARD)
            nloc = np.arange(SHARD)
            rr = rank[nloc]
            rowid = (rr % 128) * COLS + rr // 128
            acc16 = (acc16 + parts[k][rowid]).astype(np.float16)
        swv = acc16[:, 64].astype(np.float32)
        z = (swv == 0.0).astype(np.float32)
        rec = 1.0 / (swv + z)
        nzm = 1.0 - z
        mean = (acc16[:, 0:64].astype(np.float32)
                * rec[:, None]).astype(np.float16).astype(np.float32)
        pd16 = pdq[:, 0:64].astype(np.float16).astype(np.float32)
        df = ((pd16 - mean).astype(np.float16).astype(np.float32)
              * nzm[:, None]).astype(np.float16).astype(np.float32)
        resv = np.where(df > 0, df, np.expm1(np.minimum(df, 0)))
        nn = min(SHARD, N - c * SHARD)
        out[c * SHARD:c * SHARD + nn] = resv[:nn]
    return out


if __name__ == "__main__":
    d = np.load("/root/problem/refdata.npz")
    o = kernel(d["h_src"], d["h_dst"], d["W_fc"], d["w_attn"],
               d["src"], d["dst"])
    exp = d["expected"]
    rel = np.linalg.norm(o - exp) / np.linalg.norm(exp)
    print(f"rel_l2 = {rel:.3e}")


# revision 6
# speedup vs baseline: 3.3510x; 1.5487x over previous
"""Trainium2 Bass kernel for nn_DiffAttention — node-major 4-pass dma_gather.

Math (edge i: src s -> dst n, per-dst softmax over incoming edges):
  p = h @ W_fc.T ; q = p @ w_attn ; w_i = exp(tanh(q_dst[n] - q_src[s]))
  out[n] = elu(p_dst[n] - (sum_i w_i p_src[s_i]) / (sum_i w_i))
(e = tanh(..) in [-1,1] so softmax max-subtraction is unnecessary.)

Device strategy (8 cores, SPMD, dst-sharded 12544 nodes/core):
  - fp16 src table [100352, 128]: rows [p(64)|one|q|junk], node id permuted
    into 4 chunks of 25088 rows (25087 real + 1 zero dummy) so every
    dma_gather idx fits in int16. Built sharded on PE, AllGather'd.
  - 4 passes per core: pass k covers edges with src in chunk k. Nodes are
    re-sorted by pass-degree; groups of <=8 subwindows x 128 nodes share a
    uniform per-node slot count F. One dma_gather per group fetches all
    edge rows node-major: slot j -> partition j%128, block j//128 = (sub,f).
    Per-edge w on ACT (qd is per-partition!), weighted rows by in-place DVE
    mult, per-node sums by DVE reduce along f. Partials [swp|sw] -> fp16
    tables in pass order.
  - Combine: per pass one dma_gather re-orders partials to canonical node
    layout (p=n//98, col=n%98); sum, then batched epilogue
    elu(p_dst - swp/sw) with zero-degree masking; one plain DMA out.
Host does only index prep (degree sorts, idx arrays, permuted h copies).
"""
import sys
sys.path.insert(0, "/opt/trn_rl_repo")
import numpy as np

N = 100000
D = 64
NC = 8
SHARD = 12544            # nodes per core = 128 * 98
COLS = 98
RCH = 25087              # real nodes per chunk
CROWS = 25088            # table rows per chunk (last row zero dummy)
NPAD = CROWS * 4         # 100352
NCHUNK = 4
ELEM = 128               # fp16 elems per src-table row (256B)
MAXSLOT = 32             # max nsub*F per gather group (pay tile 8KB/part)
MAXSUB = 8
MAIN_REPEAT = 1


# ---------------------------------------------------------------- host prep
def _wrap_idx(flat):
    """[n] int -> [128, n//16] int16, idx j at [16s + j%16, j//16] stripes
    replicated (HW SWDGE reads stripe 16:32; interp reads 0:16)."""
    w = flat.reshape(-1, 16).T
    return np.ascontiguousarray(np.tile(w, (8, 1)).astype(np.int16))


def _prep(src, dst):
    src = np.asarray(src, np.int64)
    dst = np.asarray(dst, np.int64)
    if np.any(np.diff(dst) < 0):
        o = np.argsort(dst, kind="stable")
        src, dst = src[o], dst[o]
    per_core = []
    for c in range(NC):
        n_lo = c * SHARD
        e_lo = np.searchsorted(dst, n_lo)
        e_hi = np.searchsorted(dst, min(n_lo + SHARD, N))
        s = src[e_lo:e_hi]
        d = dst[e_lo:e_hi] - n_lo
        passes = []
        for k in range(NCHUNK):
            m = (s // RCH) == k
            sk = (s[m] % RCH).astype(np.int64)
            dk = d[m]
            deg = np.bincount(dk, minlength=SHARD)
            order = np.argsort(-deg, kind="stable")
            rank = np.empty(SHARD, np.int64)
            rank[order] = np.arange(SHARD)
            eo = np.argsort(rank[dk], kind="stable")
            sk = sk[eo]
            cnt = deg[order]
            starts = np.concatenate([[0], np.cumsum(cnt)])
            passes.append(dict(cnt=cnt, starts=starts, sk=sk,
                               order=order, rank=rank))
        per_core.append(passes)

    # cross-core per-subwindow max degree -> shared group schema per pass
    schema = []   # per pass: list of (sw0, nsub, F)
    for k in range(NCHUNK):
        fsub = np.zeros(COLS, np.int64)
        for c in range(NC):
            cnt = per_core[c][k]["cnt"]
            fsub = np.maximum(fsub, cnt[0:SHARD:128])
        groups = []
        sw = 0
        while sw < COLS:
            f_g = int(fsub[sw])
            if f_g == 0:
                groups.append((sw, COLS - sw, 0))
                break
            nsub = 1
            while (sw + nsub < COLS and nsub < MAXSUB
                   and (nsub + 1) * int(fsub[sw]) <= MAXSLOT):
                nsub += 1
            groups.append((sw, nsub, f_g))
            sw += nsub
        schema.append(groups)

    cores = []
    for c in range(NC):
        gidx, qidx, cidx = [], [], []
        for k in range(NCHUNK):
            P = per_core[c][k]
            cols_k = []
            for (sw0, nsub, f_g) in schema[k]:
                if f_g == 0:
                    continue
                ni = 128 * nsub * f_g
                j = np.arange(ni)
                p = j % 128
                b = j // 128
                r = (sw0 + b // f_g) * 128 + p
                f = b % f_g
                idxf = np.full(ni, RCH, np.int64)      # chunk dummy row
                valid = f < P["cnt"][r]
                ei = P["starts"][r] + f
                idxf[valid] = P["sk"][ei[valid]]
                cols_k.append(_wrap_idx(idxf))
            gidx.append(np.concatenate(cols_k, axis=1) if cols_k
                        else np.zeros((128, 0), np.int16))
            qidx.append(_wrap_idx(P["order"]))
            jc = np.arange(SHARD)
            nloc = (jc % 128) * COLS + jc // 128
            rr = P["rank"][nloc]
            cidx.append(_wrap_idx((rr % 128) * COLS + rr // 128))
        cores.append(dict(
            gidx=np.concatenate(gidx, axis=1),
            qidx=np.stack(qidx), cidx=np.stack(cidx)))
    return schema, cores


def _perm_h_src(h):
    hp = np.zeros((NPAD, D + 1), np.float32)
    r = np.arange(NPAD)
    rin = r % CROWS
    n = (r // CROWS) * RCH + rin
    real = (rin < RCH) & (n < N)
    hp[real, :D] = h[n[real]]
    hp[real, D] = 1.0
    return hp


def _local_h_dst(h, c):
    hp = np.zeros((SHARD, D + 1), np.float32)
    n_lo = c * SHARD
    nn = min(SHARD, N - n_lo)
    hp[:nn, :D] = h[n_lo:n_lo + nn]
    hp[:nn, D] = 1.0
    return hp


def _blockT(x):   # [12544, 65] -> [98, 65, 128]
    return np.ascontiguousarray(x.reshape(COLS, 128, D + 1).transpose(0, 2, 1))


def _mkM(W, wat):
    M = np.zeros((D + 1, 66), np.float32)
    M[:D, :D] = W.T
    M[D, D] = 1.0
    M[:D, 65] = W.T @ wat
    return M


# ---------------------------------------------------------------- device
def _build_program(schema, gcols, repeat):
    from concourse import bass, bacc, mybir, tile
    from concourse.library_config import mlp as mlp_lib
    f32, f16, i16 = mybir.dt.float32, mybir.dt.float16, mybir.dt.int16
    FN = mybir.ActivationFunctionType
    OP = mybir.AluOpType

    SLOT = max([MAXSLOT] + [n * f for p in schema for (_, n, f) in p])
    nc = bacc.Bacc("TRN2", target_bir_lowering=False, debug=False,
                   num_devices=NC, num_swdge_queues=4)
    hsT_e = nc.dram_tensor("hsT", [COLS, D + 1, 128], f32,
                           kind="ExternalInput")
    hdT_e = nc.dram_tensor("hdT", [COLS, D + 1, 128], f32,
                           kind="ExternalInput")
    mm_e = nc.dram_tensor("mm", [D + 1, 66], f32, kind="ExternalInput")
    gidx_e = nc.dram_tensor("gidx", [128, gcols], i16, kind="ExternalInput")
    qidx_e = nc.dram_tensor("qidx", [NCHUNK, 128, SHARD // 16], i16,
                            kind="ExternalInput")
    cidx_e = nc.dram_tensor("cidx", [NCHUNK, 128, SHARD // 16], i16,
                            kind="ExternalInput")
    res_e = nc.dram_tensor("res", [SHARD, D], f32, kind="ExternalOutput")

    with tile.TileContext(nc) as tc:
        with tc.tile_pool(name="cst", bufs=1) as cp, \
             tc.tile_pool(name="mn", bufs=1) as sp, \
             tc.tile_pool(name="dr", bufs=1, space="DRAM") as dp:
            bp = tc.alloc_tile_pool(name="bld", bufs=3)
            pp = tc.alloc_tile_pool(name="ps", bufs=2, space="PSUM")
            nc.gpsimd.load_library(mlp_lib)
            mm = cp.tile([D + 1, 66], f32)
            nc.sync.dma_start(out=mm[:], in_=mm_e[:])

            tsrc_sh = dp.tile([SHARD, ELEM], f16)
            tdst = dp.tile([SHARD, D], f16)
            qtab = dp.tile([SHARD, ELEM], f16)
            for b in range(COLS):
                hs = bp.tile([D + 1, 128], f32, tag="hs")
                nc.sync.dma_start(out=hs[:], in_=hsT_e[b])
                ps = pp.tile([128, 66], f32, space="PSUM", tag="ps")
                nc.tensor.matmul(out=ps[:], lhsT=hs[:], rhs=mm[:],
                                 start=True, stop=True)
                t16 = bp.tile([128, ELEM], f16, tag="t16")
                nc.vector.tensor_copy(t16[:, 0:66], ps[:])
                nc.sync.dma_start(out=tsrc_sh[b * 128:(b + 1) * 128, :],
                                  in_=t16[:])
                hd = bp.tile([D + 1, 128], f32, tag="hd")
                nc.sync.dma_start(out=hd[:], in_=hdT_e[b])
                ps2 = pp.tile([128, 66], f32, space="PSUM", tag="ps2")
                nc.tensor.matmul(out=ps2[:], lhsT=hd[:], rhs=mm[:],
                                 start=True, stop=True)
                td = bp.tile([128, D], f16, tag="td")
                nc.vector.tensor_copy(td[:], ps2[:, 0:64])
                nc.sync.dma_start(out=tdst[b * 128:(b + 1) * 128, :],
                                  in_=td[:])
                q16 = bp.tile([128, ELEM], f16, tag="q16")
                nc.scalar.activation(out=q16[:, 0:1], in_=ps2[:, 65:66],
                                     func=FN.Copy)
                nc.sync.dma_start(out=qtab[b * 128:(b + 1) * 128, :],
                                  in_=q16[:])

            tsrc = dp.tile([NPAD, ELEM], f16)
            nc.gpsimd.collective_compute(
                "AllGather", OP.bypass, replica_groups=[list(range(NC))],
                ins=[tsrc_sh.opt()], outs=[tsrc.opt()])

            # qd per pass in rank layout: [128, NCHUNK*COLS] f32
            qd = cp.tile([128, NCHUNK * COLS], f32)
            QSPL = [24, 24, 25, 25]     # col-blocks per sub-gather
            for k in range(NCHUNK):
                qi = bp.tile([128, SHARD // 16], i16, tag="qi")
                nc.sync.dma_start(out=qi[:], in_=qidx_e[k])
                c0 = 0
                for h, nb in enumerate(QSPL):
                    nih = nb * 128
                    qrows = sp.tile([128, SLOT * ELEM], f16, tag="pay",
                                    bufs=7)
                    qv = qrows[:, :nih].rearrange("p (a b) -> p a b", b=ELEM)
                    nc.gpsimd.dma_gather(
                        qv, qtab[:], qi[:, c0 * 8:(c0 + nb) * 8],
                        nih, nih, ELEM, single_packet=False,
                        queue_num=h % 4)
                    nc.vector.tensor_copy(
                        qd[:, k * COLS + c0:k * COLS + c0 + nb],
                        qv[:, :, 0:1].squeeze(2))
                    c0 += nb
            ci = []
            for k in range(NCHUNK):
                t = cp.tile([128, SHARD // 16], i16, tag=f"ci{k}")
                nc.sync.dma_start(out=t[:], in_=cidx_e[k])
                ci.append(t)
            bp.release()

            parts = [dp.tile([128, COLS, ELEM], f16, name=f"part{k}")
                     for k in range(NCHUNK)]

            rep = tc.For_i(0, repeat, 1) if repeat > 1 else None
            if rep is not None:
                rep.__enter__()
            off = 0
            qrr = [0]
            acc16 = sp.tile([128, COLS, 66], f16, tag="acc16")
            for k in range(NCHUNK):
                for (sw0, nsub, f_g) in schema[k]:
                    if f_g == 0:
                        nz = nsub
                        zc = sw0
                        while nz > 0:
                            zn = min(nz, MAXSUB)
                            zt = sp.tile([128, MAXSUB, ELEM], f16, tag="zt",
                                         bufs=2)
                            nc.vector.memset(zt[:, :zn, :], 0.0)
                            nc.sync.dma_start(
                                out=parts[k][:, zc:zc + zn, :],
                                in_=zt[:, :zn, :])
                            zc += zn
                            nz -= zn
                        continue
                    nig = 128 * nsub * f_g
                    ncol = nig // 16
                    it = sp.tile([128, SLOT * 8], i16, tag="it", bufs=7)
                    nc.sync.dma_start(out=it[:, :ncol],
                                      in_=gidx_e[:, off:off + ncol])
                    off += ncol
                    pay = sp.tile([128, SLOT * ELEM], f16, tag="pay",
                                  bufs=7)
                    pay4 = pay[:, :nsub * f_g * ELEM].rearrange(
                        "p (s f e) -> p s f e", s=nsub, e=ELEM)
                    nc.gpsimd.dma_gather(
                        pay[:, :nsub * f_g * ELEM].rearrange(
                            "p (a b) -> p a b", b=ELEM),
                        tsrc[k * CROWS:(k + 1) * CROWS, :],
                        it[:, :ncol], nig, nig, ELEM, single_packet=False,
                        queue_num=qrr[0] % 4)
                    qrr[0] += 1
                    # per-edge weight: w = exp(tanh(qd - qs))
                    dif = sp.tile([128, SLOT], f32, tag="dif", bufs=3)
                    difv = dif[:, :nsub * f_g].rearrange(
                        "p (s f) -> p s f", s=nsub)
                    nc.vector.tensor_tensor(
                        difv,
                        qd[:, k * COLS + sw0:k * COLS + sw0 + nsub]
                        .unsqueeze(2).broadcast_to((128, nsub, f_g)),
                        pay4[:, :, :, 65], op=OP.subtract)
                    th = sp.tile([128, SLOT], f32, tag="th", bufs=3)
                    nc.scalar.activation(out=th[:, :nsub * f_g],
                                         in_=dif[:, :nsub * f_g],
                                         func=FN.Tanh)
                    w16 = sp.tile([128, SLOT], f16, tag="w16", bufs=3)
                    nc.scalar.activation(out=w16[:, :nsub * f_g],
                                         in_=th[:, :nsub * f_g], func=FN.Exp)
                    # weighted rows in place: pay[:,:,:,0:65] *= w
                    payT = pay4[:, :, :, 0:65].transpose([0, 1, 3, 2])
                    nc.vector.tensor_tensor(
                        payT, payT,
                        w16[:, :nsub * f_g].rearrange("p (s f) -> p s f",
                                                      s=nsub)
                        .unsqueeze(2).broadcast_to((128, nsub, 65, f_g)),
                        op=OP.mult)
                    acc = sp.tile([128, MAXSUB * 65], f32, tag="acc", bufs=3)
                    nc.vector.tensor_reduce(
                        out=acc[:, :nsub * 65].rearrange(
                            "p (s e) -> p s e", s=nsub),
                        in_=payT, axis=mybir.AxisListType.X, op=OP.add)
                    pout = sp.tile([128, MAXSUB, ELEM], f16, tag="pout",
                                   bufs=3)
                    nc.scalar.activation(
                        out=pout[:, :nsub, 0:65],
                        in_=acc[:, :nsub * 65].rearrange(
                            "p (s e) -> p s e", s=nsub), func=FN.Copy)
                    nc.sync.dma_start(out=parts[k][:, sw0:sw0 + nsub, :],
                                      in_=pout[:, :nsub, :])
                # combine pass k's partials (overlaps next pass's gathers)
                pt = sp.tile([128, COLS * ELEM], f16, tag="pt", bufs=1)
                ptv = pt[:].rearrange("p (a b) -> p a b", b=ELEM)
                nc.gpsimd.dma_gather(
                    ptv, parts[k][:].rearrange("p g e -> (p g) e"),
                    ci[k][:], SHARD, SHARD, ELEM, single_packet=False,
                    queue_num=k % 4)
                if k == 0:
                    nc.vector.tensor_copy(acc16[:], ptv[:, :, 0:66])
                else:
                    nc.vector.tensor_tensor(acc16[:], acc16[:],
                                            ptv[:, :, 0:66], op=OP.add)

            # ---- epilogue (canonical layout: node = p*98 + col)
            pd = sp.tile([128, COLS, D], f16, tag="pd")
            nc.sync.dma_start(
                out=pd[:], in_=tdst[:].rearrange("(p c) e -> p c e", p=128))
            sw_ = acc16[:, :, 64:65].squeeze(2)
            z = sp.tile([128, COLS], f32, tag="z")
            nc.vector.tensor_scalar(out=z[:], in0=sw_, scalar1=0.0,
                                    scalar2=None, op0=OP.is_equal)
            den = sp.tile([128, COLS], f32, tag="den")
            nc.vector.tensor_tensor(den[:], sw_, z[:], op=OP.add)
            rec = sp.tile([128, COLS], f32, tag="rec")
            nc.vector.reciprocal(rec[:], den[:])
            nzm = sp.tile([128, COLS], f32, tag="nzm")
            nc.vector.tensor_scalar(out=nzm[:], in0=z[:], scalar1=-1.0,
                                    scalar2=1.0, op0=OP.mult, op1=OP.add)
            mean = sp.tile([128, COLS, D], f16, tag="e16", bufs=2)
            nc.vector.tensor_tensor(
                mean[:], acc16[:, :, 0:64],
                rec[:].unsqueeze(2).broadcast_to((128, COLS, D)), op=OP.mult)
            df = sp.tile([128, COLS, D], f16, tag="df")
            nc.vector.tensor_tensor(df[:], pd[:], mean[:], op=OP.subtract)
            nc.vector.tensor_tensor(
                df[:], df[:],
                nzm[:].unsqueeze(2).broadcast_to((128, COLS, D)), op=OP.mult)
            ng = sp.tile([128, COLS, D], f16, tag="e16", bufs=2)
            nc.vector.tensor_scalar(out=ng[:], in0=df[:], scalar1=0.0,
                                    scalar2=None, op0=OP.min)
            ex = sp.tile([128, COLS, D], f16, tag="e16", bufs=2)
            nc.scalar.activation(out=ex[:], in_=ng[:], func=FN.Exp)
            nc.vector.tensor_scalar(out=df[:], in0=df[:], scalar1=0.0,
                                    scalar2=None, op0=OP.max)
            resf = sp.tile([128, COLS, D], f32, tag="resf")
            nc.vector.scalar_tensor_tensor(
                out=resf[:], in0=ex[:], scalar=-1.0, in1=df[:],
                op0=OP.add, op1=OP.add)
            nc.sync.dma_start(
                out=res_e[:].rearrange("(p c) e -> p c e", p=128),
                in_=resf[:])
            if rep is not None:
                rep.__exit__(None, None, None)
            pp.release()
    nc.compile()
    return nc


_CACHE = {}


def _get_program(schema, gcols, repeat):
    key = (tuple(tuple(g) for p in schema for g in p), gcols, repeat)
    if key not in _CACHE:
        _CACHE[key] = _build_program(schema, gcols, repeat)
    return _CACHE[key]


def kernel(h_src, h_dst, W_fc, w_attn, src, dst, _main_repeat=MAIN_REPEAT):
    from concourse.bass_utils import run_bass_kernel_spmd

    h_src = np.ascontiguousarray(np.asarray(h_src, np.float32))
    h_dst = np.ascontiguousarray(np.asarray(h_dst, np.float32))
    W_fc = np.ascontiguousarray(np.asarray(W_fc, np.float32))
    w_attn = np.ascontiguousarray(np.asarray(w_attn, np.float32)).reshape(D)
    schema, cores = _prep(src, dst)
    gcols = cores[0]["gidx"].shape[1]

    hsp = _perm_h_src(h_src)
    M = _mkM(W_fc, w_attn)
    in_maps = []
    for c in range(NC):
        in_maps.append({
            "hsT": _blockT(hsp[c * SHARD:(c + 1) * SHARD]),
            "hdT": _blockT(_local_h_dst(h_dst, c)),
            "mm": M,
            "gidx": cores[c]["gidx"],
            "qidx": cores[c]["qidx"],
            "cidx": cores[c]["cidx"],
        })
    nc = _get_program(schema, gcols, _main_repeat)
    res = run_bass_kernel_spmd(nc, in_maps, list(range(NC)))

    out = np.zeros((N, D), np.float32)
    for c in range(NC):
        nn = min(SHARD, N - c * SHARD)
        out[c * SHARD:c * SHARD + nn] = res.results[c]["res"][:nn]
    return out


# ---------------------------------------------------------------- local sim
def simulate(h_src, h_dst, W_fc, w_attn, src, dst):
    """Numpy mirror of the device program (incl. fp16 quantization)."""
    h_src = np.asarray(h_src, np.float32)
    h_dst = np.asarray(h_dst, np.float32)
    W_fc = np.asarray(W_fc, np.float32)
    w_attn = np.asarray(w_attn, np.float32).reshape(D)
    schema, cores = _prep(src, dst)
    M = _mkM(W_fc, w_attn)
    hsp = _perm_h_src(h_src)
    tab16 = (hsp @ M).astype(np.float16)         # [NPAD, 66]
    out = np.zeros((N, D), np.float32)
    for c in range(NC):
        hd = _local_h_dst(h_dst, c)
        pdq = hd @ M                              # [SHARD, 66] f32
        qtab16 = pdq[:, 65].astype(np.float16)
        parts = []
        src64 = np.asarray(src, np.int64)
        dst64 = np.asarray(dst, np.int64)
        P = cores[c]
        # recompute per-pass structures (same as _prep)
        n_lo = c * SHARD
        e_lo = np.searchsorted(dst64, n_lo)
        e_hi = np.searchsorted(dst64, min(n_lo + SHARD, N))
        s_ = src64[e_lo:e_hi]
        d_ = dst64[e_lo:e_hi] - n_lo
        for k in range(NCHUNK):
            part = np.zeros((SHARD, 66), np.float16)   # row = p*98 + G
            m = (s_ // RCH) == k
            deg = np.bincount(d_[m], minlength=SHARD)
            order = np.argsort(-deg, kind="stable")
            qd_rank = qtab16[order].astype(np.float32)   # [rank]
            # decode gidx arrays back? simpler: recompute idxf same way
            sk = (s_[m] % RCH).astype(np.int64)
            rank = np.empty(SHARD, np.int64)
            rank[order] = np.arange(SHARD)
            eo = np.argsort(rank[d_[m]], kind="stable")
            sk = sk[eo]
            cnt = deg[order]
            starts = np.concatenate([[0], np.cumsum(cnt)])
            for (sw0, nsub, f_g) in schema[k]:
                if f_g == 0:
                    continue
                ni = 128 * nsub * f_g
                j = np.arange(ni)
                p = j % 128
                b = j // 128
                r = (sw0 + b // f_g) * 128 + p
                f = b % f_g
                idxf = np.full(ni, RCH, np.int64)
                valid = f < cnt[r]
                idxf[valid] = sk[(starts[r] + f)[valid]]
                rows = tab16[k * CROWS + idxf]            # [ni, 66]
                qs = rows[:, 65].astype(np.float32)
                dif = qd_rank[r] - qs
                w16 = np.exp(np.tanh(dif)).astype(np.float16)
                wp = (rows[:, 0:65] * w16[:, None]).astype(np.float16)
                acc = wp.astype(np.float32).reshape(nsub, f_g, 128, 65) \
                    .sum(axis=1)                           # [nsub? ...]
                # careful: j order is (b=(sub,f), p): reshape [(nsub f) 128]
                part_rows = acc.astype(np.float16)         # [nsub, 128, 65]
                for s2 in range(nsub):
                    G = sw0 + s2
                    part[np.arange(128) * COLS + G, 0:65] = part_rows[s2]
            parts.append(part)
        # combine in canonical layout
        acc16 = np.zeros((SHARD, 66), np.float16)
        for k in range(NCHUNK):
            m = (s_ // RCH) == k
            deg = np.bincount(d_[m], minlength=SHARD)
            rank = np.empty(SHARD, np.int64)
            rank[np.argsort(-deg, kind="stable")] = np.arange(SHARD)
            nloc = np.arange(SHARD)
            rr = rank[nloc]
            rowid = (rr % 128) * COLS + rr // 128
            acc16 = (acc16 + parts[k][rowid]).astype(np.float16)
        swv = acc16[:, 64].astype(np.float32)
        z = (swv == 0.0).astype(np.float32)
        rec = 1.0 / (swv + z)
        nzm = 1.0 - z
        mean = (acc16[:, 0:64].astype(np.float32)
                * rec[:, None]).astype(np.float16).astype(np.float32)
        pd16 = pdq[:, 0:64].astype(np.float16).astype(np.float32)
        df = ((pd16 - mean).astype(np.float16).astype(np.float32)
              * nzm[:, None]).astype(np.float16).astype(np.float32)
        resv = np.where(df > 0, df, np.expm1(np.minimum(df, 0)))
        nn = min(SHARD, N - c * SHARD)
        out[c * SHARD:c * SHARD + nn] = resv[:nn]
    return out


if __name__ == "__main__":
    d = np.load("/root/problem/refdata.npz")
    o = kernel(d["h_src"], d["h_dst"], d["W_fc"], d["w_attn"],
               d["src"], d["dst"])
    exp = d["expected"]
    rel = np.linalg.norm(o - exp) / np.linalg.norm(exp)
    print(f"rel_l2 = {rel:.3e}")


# revision 9
# speedup vs baseline: 3.7978x; 1.1333x over previous
"""Trainium2 Bass kernel for nn_DiffAttention — node-major 4-pass dma_gather.

Math (edge i: src s -> dst n, per-dst softmax over incoming edges):
  p = h @ W_fc.T ; q = p @ w_attn ; w_i = exp(tanh(q_dst[n] - q_src[s]))
  out[n] = elu(p_dst[n] - (sum_i w_i p_src[s_i]) / (sum_i w_i))
(e = tanh(..) in [-1,1] so softmax max-subtraction is unnecessary.)

Device strategy (8 cores, SPMD, dst-sharded 12544 nodes/core):
  - fp16 src table [100352, 128]: rows [p(64)|one|q|junk], node id permuted
    into 4 chunks of 25088 rows (25087 real + 1 zero dummy) so every
    dma_gather idx fits in int16. Built sharded on PE, AllGather'd.
  - 4 passes per core: pass k covers edges with src in chunk k. Nodes are
    re-sorted by pass-degree; groups of <=8 subwindows x 128 nodes share a
    uniform per-node slot count F. One dma_gather per group fetches all
    edge rows node-major: slot j -> partition j%128, block j//128 = (sub,f).
    Per-edge w on ACT (qd is per-partition!), weighted rows by in-place DVE
    mult, per-node sums by DVE reduce along f. Partials [swp|sw] -> fp16
    tables in pass order.
  - Combine: per pass one dma_gather re-orders partials to canonical node
    layout (p=n//98, col=n%98); sum, then batched epilogue
    elu(p_dst - swp/sw) with zero-degree masking; one plain DMA out.
Host does only index prep (degree sorts, idx arrays, permuted h copies).
"""
import sys
sys.path.insert(0, "/opt/trn_rl_repo")
import numpy as np

N = 100000
D = 64
NC = 8
SHARD = 12544            # nodes per core = 128 * 98
COLS = 98
RCH = 25087              # real nodes per chunk
CROWS = 25088            # table rows per chunk (last row zero dummy)
NPAD = CROWS * 4         # 100352
NCHUNK = 4
ELEM = 128               # fp16 elems per src-table row (256B)
MAXSLOT = 32             # max nsub*F per gather group (pay tile 8KB/part)
MAXSUB = 8
MAIN_REPEAT = 1


# ---------------------------------------------------------------- host prep
def _wrap_idx(flat):
    """[n] int -> [128, n//16] int16, idx j at [16s + j%16, j//16] stripes
    replicated (HW SWDGE reads stripe 16:32; interp reads 0:16)."""
    w = flat.reshape(-1, 16).T
    return np.ascontiguousarray(np.tile(w, (8, 1)).astype(np.int16))


def _prep(src, dst):
    src = np.asarray(src, np.int64)
    dst = np.asarray(dst, np.int64)
    if np.any(np.diff(dst) < 0):
        o = np.argsort(dst, kind="stable")
        src, dst = src[o], dst[o]
    per_core = []
    for c in range(NC):
        n_lo = c * SHARD
        e_lo = np.searchsorted(dst, n_lo)
        e_hi = np.searchsorted(dst, min(n_lo + SHARD, N))
        s = src[e_lo:e_hi]
        d = dst[e_lo:e_hi] - n_lo
        passes = []
        for k in range(NCHUNK):
            m = (s // RCH) == k
            sk = (s[m] % RCH).astype(np.int64)
            dk = d[m]
            deg = np.bincount(dk, minlength=SHARD)
            order = np.argsort(-deg, kind="stable")
            rank = np.empty(SHARD, np.int64)
            rank[order] = np.arange(SHARD)
            eo = np.argsort(rank[dk], kind="stable")
            sk = sk[eo]
            cnt = deg[order]
            starts = np.concatenate([[0], np.cumsum(cnt)])
            passes.append(dict(cnt=cnt, starts=starts, sk=sk,
                               order=order, rank=rank))
        per_core.append(passes)

    # cross-core per-subwindow max degree -> shared group schema per pass
    schema = []   # per pass: list of (sw0, nsub, F)
    for k in range(NCHUNK):
        fsub = np.zeros(COLS, np.int64)
        for c in range(NC):
            cnt = per_core[c][k]["cnt"]
            fsub = np.maximum(fsub, cnt[0:SHARD:128])
        groups = []
        sw = 0
        while sw < COLS:
            f_g = int(fsub[sw])
            if f_g == 0:
                groups.append((sw, COLS - sw, 0))
                break
            nsub = 1
            while (sw + nsub < COLS and nsub < MAXSUB
                   and (nsub + 1) * int(fsub[sw]) <= MAXSLOT):
                nsub += 1
            groups.append((sw, nsub, f_g))
            sw += nsub
        schema.append(groups)

    cores = []
    for c in range(NC):
        gidx, qidx, cidx = [], [], []
        for k in range(NCHUNK):
            P = per_core[c][k]
            cols_k = []
            for (sw0, nsub, f_g) in schema[k]:
                if f_g == 0:
                    continue
                ni = 128 * nsub * f_g
                j = np.arange(ni)
                p = j % 128
                b = j // 128
                r = (sw0 + b // f_g) * 128 + p
                f = b % f_g
                idxf = np.full(ni, RCH, np.int64)      # chunk dummy row
                valid = f < P["cnt"][r]
                ei = P["starts"][r] + f
                idxf[valid] = P["sk"][ei[valid]]
                cols_k.append(_wrap_idx(idxf))
            gidx.append(np.concatenate(cols_k, axis=1) if cols_k
                        else np.zeros((128, 0), np.int16))
            qidx.append(_wrap_idx(P["order"]))
            jc = np.arange(SHARD)
            nloc = (jc % 128) * COLS + jc // 128
            rr = P["rank"][nloc]
            cidx.append(_wrap_idx((rr % 128) * COLS + rr // 128))
        cores.append(dict(
            gidx=np.concatenate(gidx, axis=1),
            qidx=np.stack(qidx), cidx=np.stack(cidx)))
    return schema, cores


def _perm_h_src(h):
    hp = np.zeros((NPAD, D + 1), np.float32)
    r = np.arange(NPAD)
    rin = r % CROWS
    n = (r // CROWS) * RCH + rin
    real = (rin < RCH) & (n < N)
    hp[real, :D] = h[n[real]]
    hp[real, D] = 1.0
    return hp


def _local_h_dst(h, c):
    hp = np.zeros((SHARD, D + 1), np.float32)
    n_lo = c * SHARD
    nn = min(SHARD, N - n_lo)
    hp[:nn, :D] = h[n_lo:n_lo + nn]
    hp[:nn, D] = 1.0
    return hp


def _blockT(x):   # [12544, 65] -> [98, 65, 128]
    return np.ascontiguousarray(x.reshape(COLS, 128, D + 1).transpose(0, 2, 1))


def _mkM(W, wat):
    M = np.zeros((D + 1, 66), np.float32)
    M[:D, :D] = W.T
    M[D, D] = 1.0
    M[:D, 65] = W.T @ wat
    return M


# ---------------------------------------------------------------- device
def _build_program(schema, gcols, repeat):
    from concourse import bass, bacc, mybir, tile
    from concourse.library_config import mlp as mlp_lib
    f32, f16, i16 = mybir.dt.float32, mybir.dt.float16, mybir.dt.int16
    FN = mybir.ActivationFunctionType
    OP = mybir.AluOpType

    SLOT = max([MAXSLOT] + [n * f for p in schema for (_, n, f) in p])
    nc = bacc.Bacc("TRN2", target_bir_lowering=False, debug=False,
                   num_devices=NC, num_swdge_queues=4)
    hsT_e = nc.dram_tensor("hsT", [COLS, D + 1, 128], f32,
                           kind="ExternalInput")
    hdT_e = nc.dram_tensor("hdT", [COLS, D + 1, 128], f32,
                           kind="ExternalInput")
    mm_e = nc.dram_tensor("mm", [D + 1, 66], f32, kind="ExternalInput")
    gidx_e = nc.dram_tensor("gidx", [128, gcols], i16, kind="ExternalInput")
    qidx_e = nc.dram_tensor("qidx", [NCHUNK, 128, SHARD // 16], i16,
                            kind="ExternalInput")
    cidx_e = nc.dram_tensor("cidx", [NCHUNK, 128, SHARD // 16], i16,
                            kind="ExternalInput")
    res_e = nc.dram_tensor("res", [SHARD, D], f16, kind="ExternalOutput")

    with tile.TileContext(nc) as tc:
        with tc.tile_pool(name="cst", bufs=1) as cp, \
             tc.tile_pool(name="mn", bufs=1) as sp, \
             tc.tile_pool(name="dr", bufs=1, space="DRAM") as dp:
            bp = tc.alloc_tile_pool(name="bld", bufs=3)
            pp = tc.alloc_tile_pool(name="ps", bufs=2, space="PSUM")
            nc.gpsimd.load_library(mlp_lib)
            mm = cp.tile([D + 1, 66], f32)
            nc.sync.dma_start(out=mm[:], in_=mm_e[:])

            tsrc_sh = dp.tile([SHARD, ELEM], f16)
            tdst = dp.tile([SHARD, D], f16)
            qtab = dp.tile([SHARD, ELEM], f16)
            for b in range(COLS):
                hs = bp.tile([D + 1, 128], f32, tag="hs")
                nc.sync.dma_start(out=hs[:], in_=hsT_e[b])
                ps = pp.tile([128, 66], f32, space="PSUM", tag="ps")
                nc.tensor.matmul(out=ps[:], lhsT=hs[:], rhs=mm[:],
                                 start=True, stop=True)
                t16 = bp.tile([128, ELEM], f16, tag="t16")
                nc.vector.tensor_copy(t16[:, 0:66], ps[:])
                nc.sync.dma_start(out=tsrc_sh[b * 128:(b + 1) * 128, :],
                                  in_=t16[:])
                hd = bp.tile([D + 1, 128], f32, tag="hd")
                nc.sync.dma_start(out=hd[:], in_=hdT_e[b])
                ps2 = pp.tile([128, 66], f32, space="PSUM", tag="ps2")
                nc.tensor.matmul(out=ps2[:], lhsT=hd[:], rhs=mm[:],
                                 start=True, stop=True)
                td = bp.tile([128, D], f16, tag="td")
                nc.vector.tensor_copy(td[:], ps2[:, 0:64])
                nc.sync.dma_start(out=tdst[b * 128:(b + 1) * 128, :],
                                  in_=td[:])
                q16 = bp.tile([128, ELEM], f16, tag="q16")
                nc.scalar.activation(out=q16[:, 0:1], in_=ps2[:, 65:66],
                                     func=FN.Copy)
                nc.sync.dma_start(out=qtab[b * 128:(b + 1) * 128, :],
                                  in_=q16[:])

            tsrc = dp.tile([NPAD, ELEM], f16)
            nc.gpsimd.collective_compute(
                "AllGather", OP.bypass, replica_groups=[list(range(NC))],
                ins=[tsrc_sh.opt()], outs=[tsrc.opt()])

            # qd per pass in rank layout: [128, NCHUNK*COLS] f32
            qd = cp.tile([128, NCHUNK * COLS], f32)
            QSPL = [24, 24, 25, 25]     # col-blocks per sub-gather
            for k in range(NCHUNK):
                qi = bp.tile([128, SHARD // 16], i16, tag="qi")
                nc.sync.dma_start(out=qi[:], in_=qidx_e[k])
                c0 = 0
                for h, nb in enumerate(QSPL):
                    nih = nb * 128
                    qrows = sp.tile([128, SLOT * ELEM], f16, tag="pay",
                                    bufs=8)
                    qv = qrows[:, :nih].rearrange("p (a b) -> p a b", b=ELEM)
                    nc.gpsimd.dma_gather(
                        qv, qtab[:], qi[:, c0 * 8:(c0 + nb) * 8],
                        nih, nih, ELEM, single_packet=False,
                        queue_num=h % 4)
                    nc.vector.tensor_copy(
                        qd[:, k * COLS + c0:k * COLS + c0 + nb],
                        qv[:, :, 0:1].squeeze(2))
                    c0 += nb
            ci = []
            for k in range(NCHUNK):
                t = cp.tile([128, SHARD // 16], i16, tag=f"ci{k}")
                nc.sync.dma_start(out=t[:], in_=cidx_e[k])
                ci.append(t)
            bp.release()

            parts = [dp.tile([128, COLS, ELEM], f16, name=f"part{k}")
                     for k in range(NCHUNK)]

            rep = tc.For_i(0, repeat, 1) if repeat > 1 else None
            if rep is not None:
                rep.__enter__()
            off = 0
            qbytes = [0, 0, 0, 0]
            acc16 = sp.tile([128, COLS, 66], f16, tag="acc16")
            for k in range(NCHUNK):
                for (sw0, nsub, f_g) in schema[k]:
                    if f_g == 0:
                        nz = nsub
                        zc = sw0
                        while nz > 0:
                            zn = min(nz, MAXSUB)
                            zt = sp.tile([128, MAXSUB, ELEM], f16, tag="zt",
                                         bufs=2)
                            nc.vector.memset(zt[:, :zn, :], 0.0)
                            nc.sync.dma_start(
                                out=parts[k][:, zc:zc + zn, :],
                                in_=zt[:, :zn, :])
                            zc += zn
                            nz -= zn
                        continue
                    nig = 128 * nsub * f_g
                    ncol = nig // 16
                    it = sp.tile([128, SLOT * 8], i16, tag="it", bufs=7)
                    nc.sync.dma_start(out=it[:, :ncol],
                                      in_=gidx_e[:, off:off + ncol])
                    off += ncol
                    pay = sp.tile([128, SLOT * ELEM], f16, tag="pay",
                                  bufs=8)
                    pay4 = pay[:, :nsub * f_g * ELEM].rearrange(
                        "p (s f e) -> p s f e", s=nsub, e=ELEM)
                    qsel = min(range(4), key=qbytes.__getitem__)
                    qbytes[qsel] += nig
                    nc.gpsimd.dma_gather(
                        pay[:, :nsub * f_g * ELEM].rearrange(
                            "p (a b) -> p a b", b=ELEM),
                        tsrc[k * CROWS:(k + 1) * CROWS, :],
                        it[:, :ncol], nig, nig, ELEM, single_packet=False,
                        queue_num=qsel)
                    # per-edge weight: w = exp(tanh(qd - qs))
                    dif = sp.tile([128, SLOT], f32, tag="dif", bufs=3)
                    difv = dif[:, :nsub * f_g].rearrange(
                        "p (s f) -> p s f", s=nsub)
                    nc.vector.tensor_tensor(
                        difv,
                        qd[:, k * COLS + sw0:k * COLS + sw0 + nsub]
                        .unsqueeze(2).broadcast_to((128, nsub, f_g)),
                        pay4[:, :, :, 65], op=OP.subtract)
                    th = sp.tile([128, SLOT], f32, tag="th", bufs=3)
                    nc.scalar.activation(out=th[:, :nsub * f_g],
                                         in_=dif[:, :nsub * f_g],
                                         func=FN.Tanh)
                    w16 = sp.tile([128, SLOT], f16, tag="w16", bufs=3)
                    nc.scalar.activation(out=w16[:, :nsub * f_g],
                                         in_=th[:, :nsub * f_g], func=FN.Exp)
                    # weighted rows in place: pay[:,:,:,0:65] *= w
                    payT = pay4[:, :, :, 0:65].transpose([0, 1, 3, 2])
                    nc.vector.tensor_tensor(
                        payT, payT,
                        w16[:, :nsub * f_g].rearrange("p (s f) -> p s f",
                                                      s=nsub)
                        .unsqueeze(2).broadcast_to((128, nsub, 65, f_g)),
                        op=OP.mult)
                    acc = sp.tile([128, MAXSUB * 65], f32, tag="acc", bufs=3)
                    nc.vector.tensor_reduce(
                        out=acc[:, :nsub * 65].rearrange(
                            "p (s e) -> p s e", s=nsub),
                        in_=payT, axis=mybir.AxisListType.X, op=OP.add)
                    pout = sp.tile([128, MAXSUB, ELEM], f16, tag="pout",
                                   bufs=3)
                    nc.scalar.activation(
                        out=pout[:, :nsub, 0:65],
                        in_=acc[:, :nsub * 65].rearrange(
                            "p (s e) -> p s e", s=nsub), func=FN.Copy)
                    nc.sync.dma_start(out=parts[k][:, sw0:sw0 + nsub, :],
                                      in_=pout[:, :nsub, :])
                # combine pass k's partials (overlaps next pass's gathers)
                pt = sp.tile([128, COLS * ELEM], f16, tag="pt", bufs=1)
                ptv = pt[:].rearrange("p (a b) -> p a b", b=ELEM)
                nc.gpsimd.dma_gather(
                    ptv, parts[k][:].rearrange("p g e -> (p g) e"),
                    ci[k][:], SHARD, SHARD, ELEM, single_packet=False,
                    queue_num=k % 4)
                if k == 0:
                    nc.vector.tensor_copy(acc16[:], ptv[:, :, 0:66])
                else:
                    nc.vector.tensor_tensor(acc16[:], acc16[:],
                                            ptv[:, :, 0:66], op=OP.add)

            # ---- epilogue (canonical layout: node = p*98 + col)
            pd = sp.tile([128, COLS, D], f16, tag="pd")
            nc.sync.dma_start(
                out=pd[:], in_=tdst[:].rearrange("(p c) e -> p c e", p=128))
            sw_ = acc16[:, :, 64:65].squeeze(2)
            z = sp.tile([128, COLS], f32, tag="z")
            nc.vector.tensor_scalar(out=z[:], in0=sw_, scalar1=0.0,
                                    scalar2=None, op0=OP.is_equal)
            den = sp.tile([128, COLS], f32, tag="den")
            nc.vector.tensor_tensor(den[:], sw_, z[:], op=OP.add)
            rec = sp.tile([128, COLS], f32, tag="rec")
            nc.vector.reciprocal(rec[:], den[:])
            nzm = sp.tile([128, COLS], f32, tag="nzm")
            nc.vector.tensor_scalar(out=nzm[:], in0=z[:], scalar1=-1.0,
                                    scalar2=1.0, op0=OP.mult, op1=OP.add)
            mean = sp.tile([128, COLS, D], f16, tag="e16", bufs=2)
            nc.vector.tensor_tensor(
                mean[:], acc16[:, :, 0:64],
                rec[:].unsqueeze(2).broadcast_to((128, COLS, D)), op=OP.mult)
            df = sp.tile([128, COLS, D], f16, tag="df")
            nc.vector.tensor_tensor(df[:], pd[:], mean[:], op=OP.subtract)
            nc.vector.tensor_tensor(
                df[:], df[:],
                nzm[:].unsqueeze(2).broadcast_to((128, COLS, D)), op=OP.mult)
            ng = sp.tile([128, COLS, D], f16, tag="e16", bufs=2)
            nc.vector.tensor_scalar(out=ng[:], in0=df[:], scalar1=0.0,
                                    scalar2=None, op0=OP.min)
            ex = sp.tile([128, COLS, D], f16, tag="e16", bufs=2)
            nc.scalar.activation(out=ex[:], in_=ng[:], func=FN.Exp)
            nc.vector.tensor_scalar(out=df[:], in0=df[:], scalar1=0.0,
                                    scalar2=None, op0=OP.max)
            resf = sp.tile([128, COLS, D], f16, tag="resf")
            nc.vector.scalar_tensor_tensor(
                out=resf[:], in0=ex[:], scalar=-1.0, in1=df[:],
                op0=OP.add, op1=OP.add)
            nc.sync.dma_start(
                out=res_e[:].rearrange("(p c) e -> p c e", p=128),
                in_=resf[:])
            if rep is not None:
                rep.__exit__(None, None, None)
            pp.release()
    nc.compile()
    return nc


_CACHE = {}


def _get_program(schema, gcols, repeat):
    key = (tuple(tuple(g) for p in schema for g in p), gcols, repeat)
    if key not in _CACHE:
        _CACHE[key] = _build_program(schema, gcols, repeat)
    return _CACHE[key]


def kernel(h_src, h_dst, W_fc, w_attn, src, dst, _main_repeat=MAIN_REPEAT):
    from concourse.bass_utils import run_bass_kernel_spmd

    h_src = np.ascontiguousarray(np.asarray(h_src, np.float32))
    h_dst = np.ascontiguousarray(np.asarray(h_dst, np.float32))
    W_fc = np.ascontiguousarray(np.asarray(W_fc, np.float32))
    w_attn = np.ascontiguousarray(np.asarray(w_attn, np.float32)).reshape(D)
    schema, cores = _prep(src, dst)
    gcols = cores[0]["gidx"].shape[1]

    hsp = _perm_h_src(h_src)
    M = _mkM(W_fc, w_attn)
    in_maps = []
    for c in range(NC):
        in_maps.append({
            "hsT": _blockT(hsp[c * SHARD:(c + 1) * SHARD]),
            "hdT": _blockT(_local_h_dst(h_dst, c)),
            "mm": M,
            "gidx": cores[c]["gidx"],
            "qidx": cores[c]["qidx"],
            "cidx": cores[c]["cidx"],
        })
    nc = _get_program(schema, gcols, _main_repeat)
    res = run_bass_kernel_spmd(nc, in_maps, list(range(NC)))

    out = np.zeros((N, D), np.float32)
    for c in range(NC):
        nn = min(SHARD, N - c * SHARD)
        out[c * SHARD:c * SHARD + nn] = res.results[c]["res"][:nn]
    return out


# ---------------------------------------------------------------- local sim
def simulate(h_src, h_dst, W_fc, w_attn, src, dst):
    """Numpy mirror of the device program (incl. fp16 quantization)."""
    h_src = np.asarray(h_src, np.float32)
    h_dst = np.asarray(h_dst, np.float32)
    W_fc = np.asarray(W_fc, np.float32)
    w_attn = np.asarray(w_attn, np.float32).reshape(D)
    schema, cores = _prep(src, dst)
    M = _mkM(W_fc, w_attn)
    hsp = _perm_h_src(h_src)
    tab16 = (hsp @ M).astype(np.float16)         # [NPAD, 66]
    out = np.zeros((N, D), np.float32)
    for c in range(NC):
        hd = _local_h_dst(h_dst, c)
        pdq = hd @ M                              # [SHARD, 66] f32
        qtab16 = pdq[:, 65].astype(np.float16)
        parts = []
        src64 = np.asarray(src, np.int64)
        dst64 = np.asarray(dst, np.int64)
        P = cores[c]
        # recompute per-pass structures (same as _prep)
        n_lo = c * SHARD
        e_lo = np.searchsorted(dst64, n_lo)
        e_hi = np.searchsorted(dst64, min(n_lo + SHARD, N))
        s_ = src64[e_lo:e_hi]
        d_ = dst64[e_lo:e_hi] - n_lo
        for k in range(NCHUNK):
            part = np.zeros((SHARD, 66), np.float16)   # row = p*98 + G
            m = (s_ // RCH) == k
            deg = np.bincount(d_[m], minlength=SHARD)
            order = np.argsort(-deg, kind="stable")
            qd_rank = qtab16[order].astype(np.float32)   # [rank]
            # decode gidx arrays back? simpler: recompute idxf same way
            sk = (s_[m] % RCH).astype(np.int64)
            rank = np.empty(SHARD, np.int64)
            rank[order] = np.arange(SHARD)
            eo = np.argsort(rank[d_[m]], kind="stable")
            sk = sk[eo]
            cnt = deg[order]
            starts = np.concatenate([[0], np.cumsum(cnt)])
            for (sw0, nsub, f_g) in schema[k]:
                if f_g == 0:
                    continue
                ni = 128 * nsub * f_g
                j = np.arange(ni)
                p = j % 128
                b = j // 128
                r = (sw0 + b // f_g) * 128 + p
                f = b % f_g
                idxf = np.full(ni, RCH, np.int64)
                valid = f < cnt[r]
                idxf[valid] = sk[(starts[r] + f)[valid]]
                rows = tab16[k * CROWS + idxf]            # [ni, 66]
                qs = rows[:, 65].astype(np.float32)
                dif = qd_rank[r] - qs
                w16 = np.exp(np.tanh(dif)).astype(np.float16)
                wp = (rows[:, 0:65] * w16[:, None]).astype(np.float16)
                acc = wp.astype(np.float32).reshape(nsub, f_g, 128, 65) \
                    .sum(axis=1)                           # [nsub? ...]
                # careful: j order is (b=(sub,f), p): reshape [(nsub f) 128]
                part_rows = acc.astype(np.float16)         # [nsub, 128, 65]
                for s2 in range(nsub):
                    G = sw0 + s2
                    part[np.arange(128) * COLS + G, 0:65] = part_rows[s2]
            parts.append(part)
        # combine in canonical layout
        acc16 = np.zeros((SHARD, 66), np.float16)
        for k in range(NCHUNK):
            m = (s_ // RCH) == k
            deg = np.bincount(d_[m], minlength=SHARD)
            rank = np.empty(SHARD, np.int64)
            rank[np.argsort(-deg, kind="stable")] = np.arange(SHARD)
            nloc = np.arange(SHARD)
            rr = rank[nloc]
            rowid = (rr % 128) * COLS + rr // 128
            acc16 = (acc16 + parts[k][rowid]).astype(np.float16)
        swv = acc16[:, 64].astype(np.float32)
        z = (swv == 0.0).astype(np.float32)
        rec = 1.0 / (swv + z)
        nzm = 1.0 - z
        mean = (acc16[:, 0:64].astype(np.float32)
                * rec[:, None]).astype(np.float16).astype(np.float32)
        pd16 = pdq[:, 0:64].astype(np.float16).astype(np.float32)
        df = ((pd16 - mean).astype(np.float16).astype(np.float32)
              * nzm[:, None]).astype(np.float16).astype(np.float32)
        resv = np.where(df > 0, df, np.expm1(np.minimum(df, 0)))
        nn = min(SHARD, N - c * SHARD)
        out[c * SHARD:c * SHARD + nn] = resv[:nn]
    return out


if __name__ == "__main__":
    d = np.load("/root/problem/refdata.npz")
    o = kernel(d["h_src"], d["h_dst"], d["W_fc"], d["w_attn"],
               d["src"], d["dst"])
    exp = d["expected"]
    rel = np.linalg.norm(o - exp) / np.linalg.norm(exp)
    print(f"rel_l2 = {rel:.3e}")
